# revision 1
# baseline (speedup 1.0000x reference)
"""GCN encoder (3x GCNConv + mean-pool + MLP) as an 8-core Trainium2 Bass kernel.

v2: minimizes per-exec input bytes and device time.

Sharding: nodes/edges partitioned by destination-node owner (8 shards).
Tables are W-premultiplied: tab0 = X@W0 (computed on device from per-core
transposed x shards, AllGathered), tab_{l+1} = relu(agg_l + b_l) @ W_{l+1}.
Per layer: per-edge source rows are gathered from the table (fp16 DRAM) with
dma_gather, scatter-added into per-destination sums via PE matmul against a
one-hot selection matrix built on DVE from compact fp16 metadata. The psum
drain fuses bias+relu on the ACT engine in feature-major layout, and the
next-table matmul transposes to node-major for free. Final layer transposes
via PE for the mean-pool one-hot matmul; pooled sums are AllReduced; the MLP
is sharded over the hidden dim with a ReduceScatter of output partials, and
each core returns only its 32-graph slice of the output.
"""

import numpy as np

NCORES = 8
F = 128            # hidden width
G = 256            # number of graphs
NH = 512           # MLP hidden
NO = 256           # MLP out
CH = 128           # edges per chunk
BATCH_CH = 32      # chunks per dma_gather batch
WINW = 256         # dst nodes per PSUM accumulation window
XT_FP8 = True      # ship x shards as fp8e4m3 (halves xt upload)
SELF_LOCAL = True  # self-loop contributions from local SBUF tiles, not gather

_cache = {}


def _host_prep(x, edge_index, batch, W0, b0, W1, b1, W2, b2, Wm1, bm1, Wm2, bm2):
    N = x.shape[0]
    FI = x.shape[1]
    SH = -(-N // (NCORES * 128)) * 128      # shard size (nodes), 128-multiple
    NP = SH * NCORES
    TILES = SH // 128
    NWIN = -(-SH // WINW)
    LO = min(32768, NP)
    HI = NP - LO
    NHS = NH // NCORES                      # MLP hidden slice per core
    GS = G // NCORES                        # output graphs per core

    if SELF_LOCAL:
        src = np.asarray(edge_index[0], dtype=np.int64)
        dst = np.asarray(edge_index[1], dtype=np.int64)
        deg = (np.bincount(np.concatenate([dst, np.arange(N, dtype=np.int64)]),
                           minlength=N).astype(np.float32))
    else:
        src = np.concatenate([edge_index[0], np.arange(N, dtype=np.int64)])
        dst = np.concatenate([edge_index[1], np.arange(N, dtype=np.int64)])
        deg = np.bincount(dst, minlength=N).astype(np.float32)
    dis = np.where(deg > 0, 1.0 / np.sqrt(np.maximum(deg, 1.0)), 0.0).astype(np.float32)
    norm = dis[src] * dis[dst]

    # per-core edge selection, ordered by (window, class, dst)
    per_core = []
    for c in range(NCORES):
        base = c * SH
        sel = (dst >= base) & (dst < base + SH)
        es = src[sel].astype(np.int64)
        ed = (dst[sel] - base).astype(np.int64)
        en = norm[sel]
        cl = (es >= LO).astype(np.int64)
        wi = ed // WINW
        order = np.lexsort((ed, cl, wi))
        per_core.append((es[order], ed[order], en[order], cl[order], wi[order]))

    # chunk counts per (window, class), equalized across cores
    counts = np.zeros((NCORES, NWIN, 2), dtype=np.int64)
    for c in range(NCORES):
        _, _, _, cl, wi = per_core[c]
        for cls in (0, 1):
            counts[c, :, cls] = np.bincount(wi[cl == cls], minlength=NWIN)
    nch = -(-counts.max(axis=0) // CH)  # [NWIN, 2] chunks
    nch_cls = nch.sum(axis=0)          # total chunks per class
    ncht = int(nch.sum())

    # shared program schedule: windows -> list of (cls, cid)
    schedule = []
    cid_ctr = [0, 0]
    for w in range(NWIN):
        lst = []
        for cls in (0, 1):
            for _ in range(int(nch[w, cls])):
                lst.append((cls, cid_ctr[cls]))
                cid_ctr[cls] += 1
        schedule.append(lst)

    # per-core streams: compact idx [16, nch_cls*8] int16;
    # per-chunk metadata split: dst-local offsets (uint8) + edge norms (fp16)
    idx_streams = [[], []]
    dlqs, nrhs = [], []
    for c in range(NCORES):
        es, ed, en, cl, wi = per_core[c]
        idx_parts = [[], []]
        import ml_dtypes
        dlq = np.zeros((128, ncht), dtype=np.uint8)
        nrh = np.zeros((128, ncht), dtype=ml_dtypes.float8_e4m3)
        g = 0
        pos = 0
        for w in range(NWIN):
            for cls in (0, 1):
                n_e = int(counts[c, w, cls])
                tot = int(nch[w, cls]) * CH
                ge, gd, gn = es[pos:pos + n_e], ed[pos:pos + n_e], en[pos:pos + n_e]
                pos += n_e
                pad = tot - n_e
                iv = ge - (LO if cls else 0)
                iv = np.concatenate([iv, np.zeros(pad, np.int64)])
                dl = np.concatenate([gd - w * WINW, np.zeros(pad, np.int64)])
                nr = np.concatenate([gn, np.zeros(pad, np.float32)])
                idx_parts[cls].append(iv.astype(np.int16))
                for k in range(tot // CH):
                    dlq[:, g] = dl[k * CH:(k + 1) * CH].astype(np.uint8)
                    nrh[:, g] = nr[k * CH:(k + 1) * CH].astype(ml_dtypes.float8_e4m3)
                    g += 1
        assert g == ncht
        for cls in (0, 1):
            arr = (np.concatenate(idx_parts[cls]) if idx_parts[cls]
                   else np.zeros(0, np.int16))
            assert arr.size == nch_cls[cls] * CH
            if arr.size:
                wrapped = arr.reshape(-1, 16).T       # [16, nch_cls*8]
            else:
                wrapped = np.zeros((16, 8), np.int16)  # dummy
            idx_streams[cls].append(np.ascontiguousarray(wrapped))
        dlqs.append(dlq)
        nrhs.append(nrh)

    # pooling helpers
    cnt = np.bincount(batch.astype(np.int64), minlength=G).astype(np.float32)
    invc_all = (1.0 / np.maximum(cnt, 1.0))[batch.astype(np.int64)]
    selfnr_all = dis * dis
    bcols, invcs, selfnrs = [], [], []
    for c in range(NCORES):
        sl = slice(c * SH, min((c + 1) * SH, N))
        b_sh = np.zeros(SH, np.float32)
        i_sh = np.zeros(SH, np.float32)
        s_sh = np.zeros(SH, np.float32)
        nreal = max(0, min((c + 1) * SH, N) - c * SH)
        if nreal > 0:
            b_sh[:nreal] = batch[sl].astype(np.float32)
            i_sh[:nreal] = invc_all[sl].astype(np.float32)
            s_sh[:nreal] = selfnr_all[sl]
        bcols.append(np.ascontiguousarray(b_sh.reshape(TILES, 128).T))
        invcs.append(np.ascontiguousarray(i_sh.reshape(TILES, 128).T))
        selfnrs.append(np.ascontiguousarray(s_sh.reshape(TILES, 128).T))

    consts = {
        "w0": W0.astype(np.float16),                     # [FI, F]
        "w1": W1.astype(np.float16), "w2": W2.astype(np.float16),
        "bcols3": np.stack([b0, b1, b2], axis=1).astype(np.float32),  # [F, 3]
    }
    if XT_FP8:
        import ml_dtypes
        xt_np = ml_dtypes.float8_e4m3
    else:
        xt_np = np.float16
    in_maps = []
    for c in range(NCORES):
        m = dict(consts)
        lo = c * SH
        hi = min((c + 1) * SH, N)
        xt = np.zeros((FI, SH), dtype=xt_np)
        xt[:, :hi - lo] = x[lo:hi].T.astype(xt_np)
        m["xt"] = np.ascontiguousarray(xt)
        m["idxlo"] = idx_streams[0][c]
        m["idxhi"] = idx_streams[1][c]
        m["dlq"] = dlqs[c]
        m["nrh"] = nrhs[c]
        m["bcol"] = bcols[c]
        m["invc"] = invcs[c]
        m["selfnr"] = selfnrs[c]
        m["wm1s"] = np.ascontiguousarray(Wm1[:, c * NHS:(c + 1) * NHS]).astype(np.float16)
        m["wm2s"] = np.ascontiguousarray(Wm2[c * NHS:(c + 1) * NHS, :]).astype(np.float16)
        m["bm1s"] = np.ascontiguousarray(
            bm1[c * NHS:(c + 1) * NHS, None]).astype(np.float32)
        m["bm2s8"] = (bm2[None, :] / NCORES).astype(np.float16)
        in_maps.append(m)

    geom = dict(N=N, FI=FI, NP=NP, SH=SH, TILES=TILES, NWIN=NWIN, LO=LO, HI=HI,
                NHS=NHS, GS=GS, nch=nch, nch_cls=[int(v) for v in nch_cls],
                ncht=ncht, schedule=schedule)
    return geom, in_maps


class _SkipRest(Exception):
    pass


def _build_bass(geom, variant="full", gcfg=None):
    import concourse.bass as bass
    import concourse.tile as tile
    from concourse import bacc, mybir

    gcfg = dict(dict(batch=8, sp=False, nq=4, qg=True), **(gcfg or {}))
    BCH = gcfg["batch"]

    f16, f32, i16 = mybir.dt.float16, mybir.dt.float32, mybir.dt.int16
    u8 = mybir.dt.uint8
    fxt = mybir.dt.float8e4 if XT_FP8 else f16
    FI, NP, SH, TILES, NWIN = (geom["FI"], geom["NP"], geom["SH"],
                               geom["TILES"], geom["NWIN"])
    LO, HI, NHS, GS = geom["LO"], geom["HI"], geom["NHS"], geom["GS"]
    nch, nch_cls, ncht = geom["nch"], geom["nch_cls"], geom["ncht"]
    schedule = geom["schedule"]

    nc = bacc.Bacc("TRN2", target_bir_lowering=False, debug=False,
                   num_devices=NCORES, num_swdge_queues=gcfg["nq"])

    xt = nc.dram_tensor("xt", [FI, SH], fxt, kind="ExternalInput")
    idxlo = nc.dram_tensor("idxlo", [16, max(nch_cls[0] * 8, 8)], i16,
                           kind="ExternalInput")
    idxhi = nc.dram_tensor("idxhi", [16, max(nch_cls[1] * 8, 8)], i16,
                           kind="ExternalInput")
    dlq = nc.dram_tensor("dlq", [128, ncht], u8, kind="ExternalInput")
    nrh = nc.dram_tensor("nrh", [128, ncht], mybir.dt.float8e4,
                     kind="ExternalInput")
    w0 = nc.dram_tensor("w0", [FI, F], f16, kind="ExternalInput")
    w1 = nc.dram_tensor("w1", [F, F], f16, kind="ExternalInput")
    w2 = nc.dram_tensor("w2", [F, F], f16, kind="ExternalInput")
    bcols3 = nc.dram_tensor("bcols3", [F, 3], f32, kind="ExternalInput")
    wm1s = nc.dram_tensor("wm1s", [F, NHS], f16, kind="ExternalInput")
    wm2s = nc.dram_tensor("wm2s", [NHS, NO], f16, kind="ExternalInput")
    bm1s = nc.dram_tensor("bm1s", [NHS, 1], f32, kind="ExternalInput")
    bm2s8 = nc.dram_tensor("bm2s8", [1, NO], f16, kind="ExternalInput")
    bcol = nc.dram_tensor("bcol", [128, TILES], f32, kind="ExternalInput")
    invc = nc.dram_tensor("invc", [128, TILES], f32, kind="ExternalInput")
    selfnr = (nc.dram_tensor("selfnr", [128, TILES], f32, kind="ExternalInput")
              if SELF_LOCAL else None)
    out = nc.dram_tensor("out", [GS, NO], f32, kind="ExternalOutput")

    shard_d = nc.dram_tensor("shard_d", [SH, F], f16)
    tabs = [nc.dram_tensor(f"tab{l}", [NP, F], f16, addr_space="Shared")
            for l in range(3)]
    gt_in = nc.dram_tensor("gt_in", [128, G], f32)
    gt_out = nc.dram_tensor("gt_out", [128, G], f32, addr_space="Shared")
    part_d = nc.dram_tensor("part_d", [G, NO], f32)
    rs_out = nc.dram_tensor("rs_out", [GS, NO], f32)

    shb = nc.alloc_sbuf_tensor("shb", [128, TILES * F], f16)

    relu_fn = mybir.ActivationFunctionType.Relu

    import contextlib
    with tile.TileContext(nc) as tc:
        with (
            contextlib.suppress(_SkipRest),
            tc.tile_pool(name="res", bufs=1) as res,
            tc.tile_pool(name="msg", bufs=6) as msgp,
            tc.tile_pool(name="sp", bufs=6) as sp,
            tc.tile_pool(name="agg", bufs=2) as aggp,
            tc.tile_pool(name="tmp", bufs=2) as tmpp,
            tc.tile_pool(name="wps", bufs=3, space="PSUM") as wps,
            tc.tile_pool(name="hps", bufs=2, space="PSUM") as hps,
            tc.tile_pool(name="gps", bufs=1, space="PSUM") as gps,
            tc.tile_pool(name="mps", bufs=1, space="PSUM") as mps,
        ):
            # ---- resident loads ----
            def load(t_dram, shape, dtype):
                t = res.tile(shape, dtype, tag=t_dram.name)
                nc.sync.dma_start(t[:], t_dram[:])
                return t

            idx_t = []
            for cls, t_dram in ((0, idxlo), (1, idxhi)):
                w = max(nch_cls[cls] * 8, 8)
                t = res.tile([128, w], i16, tag=f"idx{cls}", name=f"idxt{cls}")
                for k in range(8):
                    nc.sync.dma_start(t[16 * k:16 * (k + 1), :], t_dram[:])
                idx_t.append(t)
            dlq_t = load(dlq, [128, ncht], u8)
            nrh_t = load(nrh, [128, ncht], mybir.dt.float8e4)
            dl32 = res.tile([128, ncht], f32, tag="dl32")
            nc.vector.tensor_copy(dl32[:], dlq_t[:])
            nr32 = res.tile([128, ncht], f32, tag="nr32")
            nc.vector.tensor_copy(nr32[:], nrh_t[:])
            xt_t = load(xt, [FI, SH], fxt)
            w0_t = load(w0, [FI, F], f16)
            w_t = {1: load(w1, [F, F], f16), 2: load(w2, [F, F], f16)}
            bcols3_t = load(bcols3, [F, 3], f32)
            wm1s_t = load(wm1s, [F, NHS], f16)
            wm2s_t = load(wm2s, [NHS, NO], f16)
            bm1s_t = load(bm1s, [NHS, 1], f32)
            bm2s8_t = load(bm2s8, [1, NO], f16)
            bcol_t = load(bcol, [128, TILES], f32)
            invc_t = load(invc, [128, TILES], f32)
            selfnr_t = load(selfnr, [128, TILES], f32) if SELF_LOCAL else None

            # iota [128, G] fp16 (values 0..G-1 per row), built on device
            io16 = res.tile([128, G], i16, tag="io16")
            nc.gpsimd.iota(io16[:], pattern=[[1, G]], base=0,
                           channel_multiplier=0)
            iota_t = res.tile([128, G], f16, tag="iota")
            nc.vector.tensor_copy(iota_t[:], io16[:])
            # identity [128,128] fp16 for PE transpose
            icol16 = res.tile([128, 1], i16, tag="icol16")
            nc.gpsimd.iota(icol16[:], pattern=[[0, 1]], base=0,
                           channel_multiplier=1)
            icolf = res.tile([128, 1], f32, tag="icolf")
            nc.vector.tensor_copy(icolf[:], icol16[:])
            ident = res.tile([128, 128], f16, tag="ident")
            nc.vector.tensor_scalar(
                out=ident[:], in0=iota_t[:, 0:128],
                scalar1=icolf[:], scalar2=None,
                op0=mybir.AluOpType.is_equal)
            ones1 = res.tile([1, 128], f16, tag="ones1")
            nc.vector.memset(ones1[:], 1.0)

            # ---- T0 = X @ W0 (per-shard), node-major into shb ----
            for t in range(TILES):
                t0p = hps.tile([128, F], f32, tag="hp")
                nc.tensor.matmul(out=t0p[:], lhsT=xt_t[:, 128 * t:128 * (t + 1)],
                                 rhs=w0_t[:], start=True, stop=True)
                nc.scalar.copy(out=shb[:, t * F:(t + 1) * F], in_=t0p[:])
            nc.sync.dma_start(
                shard_d.ap().rearrange("(t p) f -> p t f", p=128),
                shb[:, :].rearrange("p (t f) -> p t f", f=F))
            if variant not in ("nocc",):
                nc.gpsimd.collective_compute(
                    "AllGather", mybir.AluOpType.bypass,
                    replica_groups=[list(range(NCORES))],
                    ins=[shard_d[:].opt()], outs=[tabs[0][:].opt()])

            # ---- 3 GCN layers ----
            gctr = [0]  # global gather counter for queue round-robin
            for l in range(3):
                tbl = tabs[l]
                tbl_ap = [tbl[0:LO, :], tbl[LO:NP, :] if HI > 0 else None]
                issued = [-1, -1]
                cur = [None, None]
                g = 0
                for w in range(NWIN):
                    width = min(WINW, SH - w * WINW)
                    chunks = schedule[w]
                    ops = [("c", x) for x in chunks]
                    use_self = SELF_LOCAL and variant not in ("gatheronly", "gs")
                    if use_self:
                        selfops = [("s", sub) for sub in range(width // 128)]
                        ops = (ops[:1] + selfops + ops[1:]) if ops else selfops
                    ps = wps.tile([128, WINW], f32, tag="wps")
                    no_chunks = not chunks
                    for j, op in enumerate(ops):
                        first, last = (j == 0), (j == len(ops) - 1)
                        if op[0] == "s":
                            sub = op[1]
                            t_idx = w * (WINW // 128) + sub
                            Sd = sp.tile([128, WINW], f16, tag="S")
                            nc.vector.tensor_scalar(
                                out=Sd[:, :128], in0=iota_t[:, :128],
                                scalar1=icolf[:],
                                scalar2=selfnr_t[:, t_idx:t_idx + 1],
                                op0=mybir.AluOpType.is_equal,
                                op1=mybir.AluOpType.mult)
                            nc.tensor.matmul(
                                out=ps[:, sub * 128:(sub + 1) * 128],
                                lhsT=shb[:, t_idx * F:(t_idx + 1) * F],
                                rhs=Sd[:, :128],
                                start=(True if no_chunks else False),
                                stop=last, skip_group_check=True)
                            continue
                        cls, cid = op[1]
                        b, slab = divmod(cid, BCH)
                        if b != issued[cls]:
                            nb = min(BCH, nch_cls[cls] - b * BCH)
                            mt = msgp.tile([128, BCH, F], f16, tag=f"msg{cls}")
                            if variant == "memset":
                                nc.vector.memset(mt[:, :nb, :], 0.0)
                            else:
                                qn = ((gctr[0] if gcfg["qg"] else b)
                                      % gcfg["nq"])
                                gctr[0] += 1
                                nc.gpsimd.dma_gather(
                                    mt[:, :nb, :], tbl_ap[cls],
                                    idx_t[cls][:, b * (BCH * 8):
                                               b * (BCH * 8) + nb * 8],
                                    nb * CH, nb * CH, F,
                                    single_packet=gcfg["sp"],
                                    queue_num=qn)
                            issued[cls] = b
                            cur[cls] = mt
                        if variant == "gatheronly":
                            g += 1
                            continue
                        S = sp.tile([128, WINW], f16, tag="S")
                        nc.vector.tensor_scalar(
                            out=S[:, :width], in0=iota_t[:, :width],
                            scalar1=dl32[:, g:g + 1],
                            scalar2=nr32[:, g:g + 1],
                            op0=mybir.AluOpType.is_equal,
                            op1=mybir.AluOpType.mult)
                        if variant == "gs":
                            g += 1
                            continue
                        nc.tensor.matmul(
                            out=ps[:, :width], lhsT=cur[cls][:, slab, :],
                            rhs=S[:, :width],
                            start=first, stop=last)
                        g += 1
                    if variant in ("gatheronly", "gs"):
                        continue
                    for sub in range(width // 128):
                        t_idx = w * (WINW // 128) + sub
                        pslice = ps[:, sub * 128:(sub + 1) * 128]
                        dst_sl = shb[:, t_idx * F:(t_idx + 1) * F]
                        if l < 2:
                            # relu(agg + b_l) in feature-major, then @W_{l+1}
                            # transposes to node-major for the next table
                            aggT = aggp.tile([128, 128], f16, tag="aggT")
                            nc.scalar.activation(
                                out=aggT[:], in_=pslice, func=relu_fn,
                                bias=bcols3_t[:, l:l + 1])
                            if variant == "gsm":
                                continue
                            tp = hps.tile([128, F], f32, tag="hp")
                            nc.tensor.matmul(out=tp[:], lhsT=aggT[:],
                                             rhs=w_t[l + 1][:],
                                             start=True, stop=True)
                            nc.scalar.copy(out=dst_sl, in_=tp[:])
                        else:
                            # H3^T = agg + b2 (no relu), transpose to
                            # node-major for pooling
                            aggT = aggp.tile([128, 128], f16, tag="aggT")
                            nc.scalar.add(out=aggT[:], in_=pslice,
                                          add=bcols3_t[:, 2:3])
                            if variant == "gsm":
                                continue
                            tp = hps.tile([128, 128], f16, tag="hp",
                                          name="tppose")
                            nc.tensor.transpose(out=tp[:], in_=aggT[:],
                                                identity=ident[:])
                            nc.scalar.copy(out=dst_sl, in_=tp[:])
                assert g == ncht
                if variant in ("gatheronly", "gs", "gsm"):
                    continue
                if l < 2:
                    nc.sync.dma_start(
                        shard_d.ap().rearrange("(t p) f -> p t f", p=128),
                        shb[:, :].rearrange("p (t f) -> p t f", f=F))
                    if variant not in ("nocc",):
                        nc.gpsimd.collective_compute(
                            "AllGather", mybir.AluOpType.bypass,
                            replica_groups=[list(range(NCORES))],
                            ins=[shard_d[:].opt()], outs=[tabs[l + 1][:].opt()])

            # ---- mean pool ----
            if variant in ("gatheronly", "gs", "gsm"):
                z = tmpp.tile([GS, NO], f32, tag="ot", name="zot")
                nc.vector.memset(z[:], 0.0)
                nc.sync.dma_start(out[:], z[:])
                raise _SkipRest
            gp = gps.tile([128, G], f32, tag="gp")
            for t in range(TILES):
                Gt = sp.tile([128, G], f16, tag="S")
                nc.vector.tensor_scalar(
                    out=Gt[:], in0=iota_t[:],
                    scalar1=bcol_t[:, t:t + 1], scalar2=invc_t[:, t:t + 1],
                    op0=mybir.AluOpType.is_equal, op1=mybir.AluOpType.mult)
                nc.tensor.matmul(out=gp[:], lhsT=shb[:, t * F:(t + 1) * F],
                                 rhs=Gt[:], start=(t == 0), stop=(t == TILES - 1))
            gtile = tmpp.tile([128, G], f32, tag="gtile")
            nc.scalar.copy(out=gtile[:], in_=gp[:])
            nc.sync.dma_start(gt_in[:], gtile[:])
            if variant not in ("nocc",):
                nc.gpsimd.collective_compute(
                    "AllReduce", mybir.AluOpType.add,
                    replica_groups=[list(range(NCORES))],
                    ins=[gt_in[:].opt()], outs=[gt_out[:].opt()])
            gfull = tmpp.tile([128, G], f32, tag="gfull")
            nc.sync.dma_start(gfull[:], gt_out[:])
            gt16 = tmpp.tile([128, G], f16, tag="gt16")
            nc.vector.tensor_copy(gt16[:], gfull[:])

            # ---- MLP (hidden dim sharded; partials reduce-scattered) ----
            mp = mps.tile([NHS, G], f32, tag="mp")
            nc.tensor.matmul(out=mp[:], lhsT=wm1s_t[:], rhs=gt16[:],
                             start=True, stop=True)
            mt16 = tmpp.tile([NHS, G], f16, tag="mt16")
            nc.scalar.activation(out=mt16[:], in_=mp[:], func=relu_fn,
                                 bias=bm1s_t[:])
            for gh in range(G // 128):
                op = mps.tile([128, NO], f32, tag="op")
                nc.tensor.matmul(
                    out=op[:], lhsT=mt16[:, 128 * gh:128 * (gh + 1)],
                    rhs=wm2s_t[:], start=True, stop=False)
                nc.tensor.matmul(
                    out=op[:], lhsT=ones1[:], rhs=bm2s8_t[:],
                    start=False, stop=True)
                ot = tmpp.tile([128, NO], f32, tag="ot")
                nc.scalar.copy(out=ot[:], in_=op[:])
                nc.sync.dma_start(part_d[128 * gh:128 * (gh + 1), :], ot[:])
            if variant not in ("nocc",):
                nc.gpsimd.collective_compute(
                    "ReduceScatter", mybir.AluOpType.add,
                    replica_groups=[list(range(NCORES))],
                    ins=[part_d[:].opt()], outs=[rs_out[:].opt()])
            rt = tmpp.tile([GS, NO], f32, tag="rt")
            nc.sync.dma_start(rt[:], rs_out[:])
            nc.sync.dma_start(out[:], rt[:])

    nc.compile()
    return nc


def _get_built(inputs, variant="full"):
    import hashlib
    h = hashlib.sha1()
    h.update(np.ascontiguousarray(inputs["edge_index"]).tobytes())
    h.update(np.ascontiguousarray(inputs["batch"]).tobytes())
    key = (variant,
           tuple(sorted((k, v.shape, str(v.dtype)) for k, v in inputs.items())),
           h.hexdigest())
    if key not in _cache:
        geom, in_maps = _host_prep(**inputs)
        nc = _build_bass(geom, variant)
        _cache[key] = (geom, nc)
    else:
        geom, nc = _cache[key]
        _, in_maps = _host_prep(**inputs)
    return geom, nc, in_maps


def kernel(**inputs):
    inputs = {k: np.asarray(v) for k, v in inputs.items()}
    geom, nc, in_maps = _get_built(inputs)
    from concourse.bass_utils import run_bass_kernel_spmd
    res = run_bass_kernel_spmd(nc, in_maps, list(range(NCORES)))
    return np.concatenate([np.asarray(res.results[c]["out"])
                           for c in range(NCORES)], axis=0)



# revision 2
# speedup vs baseline: 1.1952x; 1.1952x over previous
"""GCN encoder (3x GCNConv + mean-pool + MLP) as an 8-core Trainium2 Bass kernel.

v3: host-precomputed scatter matrices streamed from DRAM.

Sharding: nodes/edges partitioned by destination-node owner (8 shards).
Tables are W-premultiplied: tab0 = X@W0 (computed on device from per-core
transposed x shards, AllGathered), tab_{l+1} = relu(agg_l + b_l) @ W_{l+1}.
Per layer: per-edge source rows are gathered from the table (fp16 DRAM) with
dma_gather; the one-hot scatter matrices S (identical across layers, graph-
dependent only) are precomputed on the HOST in fp8e4m3 and streamed from DRAM
in groups, so the vector engine does no per-chunk work. Scatter-add happens
via PE matmul (lhsT = gathered messages fp16, rhs = S fp8). Self-loop and
mean-pool selection matrices are host-built fp16 and SBUF-resident. The psum
drain fuses bias+relu on the ACT engine in feature-major layout, and the
next-table matmul transposes to node-major for free. Final layer transposes
via PE for the mean-pool matmul; pooled sums are AllReduced; the MLP is
sharded over the hidden dim with a ReduceScatter of output partials, and each
core returns only its 32-graph slice of the output.
"""

import numpy as np

NCORES = 8
F = 128            # hidden width
G = 256            # number of graphs
NH = 512           # MLP hidden
NO = 256           # MLP out
CH = 128           # edges per chunk
BATCH_CH = 16      # chunks per dma_gather batch
WINW = 128         # dst nodes per PSUM accumulation window
SGRP = 8           # chunks per S-matrix stream DMA
XT_FP8 = True      # ship x shards as fp8e4m3 (halves xt upload)

_cache = {}


def _host_prep(x, edge_index, batch, W0, b0, W1, b1, W2, b2, Wm1, bm1, Wm2, bm2):
    import ml_dtypes
    f8 = ml_dtypes.float8_e4m3

    N = x.shape[0]
    FI = x.shape[1]
    SH = -(-N // (NCORES * 128)) * 128      # shard size (nodes), 128-multiple
    NP = SH * NCORES
    TILES = SH // 128
    NWIN = -(-SH // WINW)
    LO = min(32768, NP)
    HI = NP - LO
    NHS = NH // NCORES                      # MLP hidden slice per core
    GS = G // NCORES                        # output graphs per core

    src = np.asarray(edge_index[0], dtype=np.int64)
    dst = np.asarray(edge_index[1], dtype=np.int64)
    deg = (np.bincount(np.concatenate([dst, np.arange(N, dtype=np.int64)]),
                       minlength=N).astype(np.float32))
    dis = np.where(deg > 0, 1.0 / np.sqrt(np.maximum(deg, 1.0)), 0.0).astype(np.float32)
    norm = dis[src] * dis[dst]

    # per-core edge selection, ordered by (window, class, dst)
    per_core = []
    for c in range(NCORES):
        base = c * SH
        sel = (dst >= base) & (dst < base + SH)
        es = src[sel].astype(np.int64)
        ed = (dst[sel] - base).astype(np.int64)
        en = norm[sel]
        cl = (es >= LO).astype(np.int64)
        wi = ed // WINW
        order = np.lexsort((ed, cl, wi))
        per_core.append((es[order], ed[order], en[order], cl[order], wi[order]))

    # chunk counts per (window, class), equalized across cores
    counts = np.zeros((NCORES, NWIN, 2), dtype=np.int64)
    for c in range(NCORES):
        _, _, _, cl, wi = per_core[c]
        for cls in (0, 1):
            counts[c, :, cls] = np.bincount(wi[cl == cls], minlength=NWIN)
    nch = -(-counts.max(axis=0) // CH)  # [NWIN, 2] chunks
    nch_cls = nch.sum(axis=0)          # total chunks per class
    ncht = int(nch.sum())

    # shared program schedule: windows -> list of (cls, cid)
    schedule = []
    cid_ctr = [0, 0]
    for w in range(NWIN):
        lst = []
        for cls in (0, 1):
            for _ in range(int(nch[w, cls])):
                lst.append((cls, cid_ctr[cls]))
                cid_ctr[cls] += 1
        schedule.append(lst)

    # per-core streams: compact idx [16, nch_cls*8] int16;
    # host-built scatter matrices s_all [128, ncht*WINW] fp8
    idx_streams = [[], []]
    s_alls = []
    for c in range(NCORES):
        es, ed, en, cl, wi = per_core[c]
        idx_parts = [[], []]
        s_all = np.zeros((128, ncht * WINW), dtype=np.float32)
        g = 0
        pos = 0
        for w in range(NWIN):
            for cls in (0, 1):
                n_e = int(counts[c, w, cls])
                tot = int(nch[w, cls]) * CH
                ge, gd, gn = es[pos:pos + n_e], ed[pos:pos + n_e], en[pos:pos + n_e]
                pos += n_e
                pad = tot - n_e
                iv = ge - (LO if cls else 0)
                iv = np.concatenate([iv, np.zeros(pad, np.int64)])
                dl = np.concatenate([gd - w * WINW, np.full(pad, -1, np.int64)])
                nr = np.concatenate([gn, np.zeros(pad, np.float32)])
                idx_parts[cls].append(iv.astype(np.int16))
                for k in range(tot // CH):
                    sl = slice(k * CH, (k + 1) * CH)
                    dlk, nrk = dl[sl], nr[sl]
                    valid = dlk >= 0
                    rows = np.nonzero(valid)[0]
                    s_all[rows, g * WINW + dlk[valid]] = nrk[valid]
                    g += 1
        assert g == ncht
        s_alls.append(s_all.astype(f8))
        for cls in (0, 1):
            arr = (np.concatenate(idx_parts[cls]) if idx_parts[cls]
                   else np.zeros(0, np.int16))
            assert arr.size == nch_cls[cls] * CH
            if arr.size:
                wrapped = arr.reshape(-1, 16).T       # [16, nch_cls*8]
            else:
                wrapped = np.zeros((16, 8), np.int16)  # dummy
            idx_streams[cls].append(np.ascontiguousarray(wrapped))

    # resident selection matrices: self-loop diag + mean-pool, per tile
    cnt = np.bincount(batch.astype(np.int64), minlength=G).astype(np.float32)
    invc_all = (1.0 / np.maximum(cnt, 1.0))[batch.astype(np.int64)]
    selfnr_all = dis * dis
    selfs_l, pools_l = [], []
    for c in range(NCORES):
        lo_n = c * SH
        hi_n = min((c + 1) * SH, N)
        nreal = max(0, hi_n - lo_n)
        selfs = np.zeros((128, TILES * 128), dtype=np.float16)
        pools = np.zeros((128, TILES * G), dtype=np.float16)
        for t in range(TILES):
            for p in range(128):
                v = t * 128 + p
                if v < nreal:
                    selfs[p, t * 128 + p] = selfnr_all[lo_n + v]
                    bgi = int(batch[lo_n + v])
                    pools[p, t * G + bgi] = invc_all[lo_n + v]
        selfs_l.append(selfs)
        pools_l.append(pools)

    consts = {
        "w0": W0.astype(np.float16),                     # [FI, F]
        "w1": W1.astype(np.float16), "w2": W2.astype(np.float16),
        "bcols3": np.stack([b0, b1, b2], axis=1).astype(np.float32),  # [F, 3]
        "ident": np.eye(128, dtype=np.float16),
    }
    xt_np = f8 if XT_FP8 else np.float16
    in_maps = []
    for c in range(NCORES):
        m = dict(consts)
        lo = c * SH
        hi = min((c + 1) * SH, N)
        xt = np.zeros((FI, SH), dtype=xt_np)
        xt[:, :hi - lo] = x[lo:hi].T.astype(xt_np)
        m["xt"] = np.ascontiguousarray(xt)
        m["idxlo"] = idx_streams[0][c]
        m["idxhi"] = idx_streams[1][c]
        m["s_all"] = s_alls[c]
        m["selfs"] = selfs_l[c]
        m["pools"] = pools_l[c]
        m["wm1s"] = np.ascontiguousarray(Wm1[:, c * NHS:(c + 1) * NHS]).astype(np.float16)
        m["wm2s"] = np.ascontiguousarray(Wm2[c * NHS:(c + 1) * NHS, :]).astype(np.float16)
        m["bm1s"] = np.ascontiguousarray(
            bm1[c * NHS:(c + 1) * NHS, None]).astype(np.float32)
        m["bm2s8"] = (bm2[None, :] / NCORES).astype(np.float16)
        in_maps.append(m)

    geom = dict(N=N, FI=FI, NP=NP, SH=SH, TILES=TILES, NWIN=NWIN, LO=LO, HI=HI,
                NHS=NHS, GS=GS, nch=nch, nch_cls=[int(v) for v in nch_cls],
                ncht=ncht, schedule=schedule)
    return geom, in_maps


def _build_bass(geom, variant="full", gcfg=None):
    import concourse.bass as bass
    import concourse.tile as tile
    from concourse import bacc, mybir

    gcfg = dict(dict(batch=BATCH_CH, sp=False, nq=4, qg=True,
                     mbufs=6, sbufs=4, wbufs=3, prep=False), **(gcfg or {}))
    BCH = gcfg["batch"]

    f16, f32, i16 = mybir.dt.float16, mybir.dt.float32, mybir.dt.int16
    f8 = mybir.dt.float8e4
    fxt = f8 if XT_FP8 else f16
    FI, NP, SH, TILES, NWIN = (geom["FI"], geom["NP"], geom["SH"],
                               geom["TILES"], geom["NWIN"])
    LO, HI, NHS, GS = geom["LO"], geom["HI"], geom["NHS"], geom["GS"]
    nch, nch_cls, ncht = geom["nch"], geom["nch_cls"], geom["ncht"]
    schedule = geom["schedule"]
    NSG = -(-ncht // SGRP)  # number of S stream groups

    nc = bacc.Bacc("TRN2", target_bir_lowering=False, debug=False,
                   num_devices=NCORES, num_swdge_queues=gcfg["nq"])

    xt = nc.dram_tensor("xt", [FI, SH], fxt, kind="ExternalInput")
    idxlo = nc.dram_tensor("idxlo", [16, max(nch_cls[0] * 8, 8)], i16,
                           kind="ExternalInput")
    idxhi = nc.dram_tensor("idxhi", [16, max(nch_cls[1] * 8, 8)], i16,
                           kind="ExternalInput")
    s_all = nc.dram_tensor("s_all", [128, ncht * WINW], f8, kind="ExternalInput")
    selfs = nc.dram_tensor("selfs", [128, TILES * 128], f16, kind="ExternalInput")
    pools = nc.dram_tensor("pools", [128, TILES * G], f16, kind="ExternalInput")
    w0 = nc.dram_tensor("w0", [FI, F], f16, kind="ExternalInput")
    w1 = nc.dram_tensor("w1", [F, F], f16, kind="ExternalInput")
    w2 = nc.dram_tensor("w2", [F, F], f16, kind="ExternalInput")
    bcols3 = nc.dram_tensor("bcols3", [F, 3], f32, kind="ExternalInput")
    ident_d = nc.dram_tensor("ident", [128, 128], f16, kind="ExternalInput")
    wm1s = nc.dram_tensor("wm1s", [F, NHS], f16, kind="ExternalInput")
    wm2s = nc.dram_tensor("wm2s", [NHS, NO], f16, kind="ExternalInput")
    bm1s = nc.dram_tensor("bm1s", [NHS, 1], f32, kind="ExternalInput")
    bm2s8 = nc.dram_tensor("bm2s8", [1, NO], f16, kind="ExternalInput")
    out = nc.dram_tensor("out", [GS, NO], f32, kind="ExternalOutput")

    shard_d = nc.dram_tensor("shard_d", [SH, F], f16)
    tabs = [nc.dram_tensor(f"tab{l}", [NP, F], f16, addr_space="Shared")
            for l in range(3)]
    gt_in = nc.dram_tensor("gt_in", [128, G], f32)
    gt_out = nc.dram_tensor("gt_out", [128, G], f32, addr_space="Shared")
    part_d = nc.dram_tensor("part_d", [G, NO], f32)
    rs_out = nc.dram_tensor("rs_out", [GS, NO], f32)

    shb = nc.alloc_sbuf_tensor("shb", [128, TILES * F], f16)

    relu_fn = mybir.ActivationFunctionType.Relu

    with tile.TileContext(nc) as tc:
        with (
            tc.tile_pool(name="res", bufs=1) as res,
            tc.tile_pool(name="msg", bufs=gcfg["mbufs"]) as msgp,
            tc.tile_pool(name="sst", bufs=gcfg["sbufs"]) as sstp,
            tc.tile_pool(name="agg", bufs=2) as aggp,
            tc.tile_pool(name="tmp", bufs=2) as tmpp,
            tc.tile_pool(name="wps", bufs=gcfg["wbufs"], space="PSUM") as wps,
            tc.tile_pool(name="hps", bufs=2, space="PSUM") as hps,
            tc.tile_pool(name="gps", bufs=1, space="PSUM") as gps,
            tc.tile_pool(name="mps", bufs=1, space="PSUM") as mps,
        ):
            # ---- resident loads ----
            def load(t_dram, shape, dtype):
                t = res.tile(shape, dtype, tag=t_dram.name)
                nc.sync.dma_start(t[:], t_dram[:])
                return t

            idx_t = []
            for cls, t_dram in ((0, idxlo), (1, idxhi)):
                w = max(nch_cls[cls] * 8, 8)
                t = res.tile([128, w], i16, tag=f"idx{cls}", name=f"idxt{cls}")
                for k in range(8):
                    nc.sync.dma_start(t[16 * k:16 * (k + 1), :], t_dram[:])
                idx_t.append(t)
            xt_t = load(xt, [FI, SH], fxt)
            selfs_t = load(selfs, [128, TILES * 128], f16)
            pools_t = load(pools, [128, TILES * G], f16)
            w0_t = load(w0, [FI, F], f16)
            w_t = {1: load(w1, [F, F], f16), 2: load(w2, [F, F], f16)}
            bcols3_t = load(bcols3, [F, 3], f32)
            ident = load(ident_d, [128, 128], f16)
            wm1s_t = load(wm1s, [F, NHS], f16)
            wm2s_t = load(wm2s, [NHS, NO], f16)
            bm1s_t = load(bm1s, [NHS, 1], f32)
            bm2s8_t = load(bm2s8, [1, NO], f16)
            ones1 = res.tile([1, 128], f16, tag="ones1")
            nc.vector.memset(ones1[:], 1.0)
            dma_sems = [nc.alloc_semaphore(f"gsem{q}")
                        for q in range(gcfg["nq"])] if gcfg["prep"] else None

            # ---- T0 = X @ W0 (per-shard), node-major into shb ----
            for t in range(TILES):
                t0p = hps.tile([128, F], f32, tag="hp")
                nc.tensor.matmul(out=t0p[:], lhsT=xt_t[:, 128 * t:128 * (t + 1)],
                                 rhs=w0_t[:], start=True, stop=True)
                nc.scalar.copy(out=shb[:, t * F:(t + 1) * F], in_=t0p[:])
            nc.sync.dma_start(
                shard_d.ap().rearrange("(t p) f -> p t f", p=128),
                shb[:, :].rearrange("p (t f) -> p t f", f=F))
            nc.gpsimd.collective_compute(
                "AllGather", mybir.AluOpType.bypass,
                replica_groups=[list(range(NCORES))],
                ins=[shard_d[:].opt()], outs=[tabs[0][:].opt()])

            # ---- 3 GCN layers ----
            gctr = [0]  # global gather counter for queue round-robin
            for l in range(3):
                tbl = tabs[l]
                tbl_ap = [tbl[0:LO, :], tbl[LO:NP, :] if HI > 0 else None]
                issued = [-1, -1]
                cur = [None, None]
                sg_cur = [-1]
                st_cur = [None]
                g = 0
                for w in range(NWIN):
                    width = min(WINW, SH - w * WINW)
                    chunks = schedule[w]
                    ops = [("c", x) for x in chunks]
                    selfop = ("s", 0)
                    ops = (ops[:1] + [selfop] + ops[1:]) if ops else [selfop]
                    ps = wps.tile([128, WINW], f32, tag="wps")
                    no_chunks = not chunks
                    for j, op in enumerate(ops):
                        first, last = (j == 0), (j == len(ops) - 1)
                        if op[0] == "s":
                            t_idx = w
                            nc.tensor.matmul(
                                out=ps[:, :width],
                                lhsT=shb[:, t_idx * F:(t_idx + 1) * F],
                                rhs=selfs_t[:, t_idx * 128:t_idx * 128 + width],
                                start=(True if no_chunks else False),
                                stop=last, skip_group_check=True)
                            continue
                        cls, cid = op[1]
                        b, slab = divmod(cid, BCH)
                        if b != issued[cls]:
                            nb = min(BCH, nch_cls[cls] - b * BCH)
                            mt = msgp.tile([128, BCH, F], f16, tag=f"msg{cls}")
                            if variant == "memset":
                                nc.vector.memset(mt[:, :nb, :], 0.0)
                            else:
                                qn = ((gctr[0] if gcfg["qg"] else b)
                                      % gcfg["nq"])
                                gctr[0] += 1
                                if gcfg["prep"]:
                                    nc.gpsimd.dma_gather(
                                        mt[:, :nb, :], tbl_ap[cls],
                                        idx_t[cls][:, b * (BCH * 8):
                                                   b * (BCH * 8) + nb * 8],
                                        nb * CH, nb * CH, F,
                                        single_packet=gcfg["sp"],
                                        prepare_only=True,
                                        sem=dma_sems[qn],
                                        queue_num=qn)
                                    nc.gpsimd.trigger_dma(count=None,
                                                          queue_num=qn)
                                else:
                                    nc.gpsimd.dma_gather(
                                        mt[:, :nb, :], tbl_ap[cls],
                                        idx_t[cls][:, b * (BCH * 8):
                                                   b * (BCH * 8) + nb * 8],
                                        nb * CH, nb * CH, F,
                                        single_packet=gcfg["sp"],
                                        queue_num=qn)
                            issued[cls] = b
                            cur[cls] = mt
                        sg = g // SGRP
                        if sg != sg_cur[0]:
                            ngc = min(SGRP, ncht - sg * SGRP)
                            st = sstp.tile([128, SGRP * WINW], f8, tag="sst")
                            nc.sync.dma_start(
                                st[:, :ngc * WINW],
                                s_all[:, sg * SGRP * WINW:
                                      (sg * SGRP + ngc) * WINW])
                            sg_cur[0] = sg
                            st_cur[0] = st
                        so = (g - sg * SGRP) * WINW
                        nc.tensor.matmul(
                            out=ps[:, :width], lhsT=cur[cls][:, slab, :],
                            rhs=st_cur[0][:, so:so + width],
                            start=first, stop=last)
                        g += 1
                    # drain window (width <= 128): fuse bias (+relu) on ACT
                    t_idx = w
                    dst_sl = shb[:, t_idx * F:(t_idx + 1) * F]
                    if l < 2:
                        aggT = aggp.tile([128, 128], f16, tag="aggT")
                        nc.scalar.activation(
                            out=aggT[:, :width], in_=ps[:, :width],
                            func=relu_fn, bias=bcols3_t[:, l:l + 1])
                        tp = hps.tile([128, F], f32, tag="hp")
                        nc.tensor.matmul(out=tp[:], lhsT=aggT[:],
                                         rhs=w_t[l + 1][:],
                                         start=True, stop=True)
                        nc.scalar.copy(out=dst_sl, in_=tp[:])
                    else:
                        aggT = aggp.tile([128, 128], f16, tag="aggT")
                        nc.scalar.add(out=aggT[:, :width], in_=ps[:, :width],
                                      add=bcols3_t[:, 2:3])
                        tp = hps.tile([128, 128], f16, tag="hp",
                                      name="tppose")
                        nc.tensor.transpose(out=tp[:], in_=aggT[:],
                                            identity=ident[:])
                        nc.scalar.copy(out=dst_sl, in_=tp[:])
                assert g == ncht
                if l < 2:
                    nc.sync.dma_start(
                        shard_d.ap().rearrange("(t p) f -> p t f", p=128),
                        shb[:, :].rearrange("p (t f) -> p t f", f=F))
                    nc.gpsimd.collective_compute(
                        "AllGather", mybir.AluOpType.bypass,
                        replica_groups=[list(range(NCORES))],
                        ins=[shard_d[:].opt()], outs=[tabs[l + 1][:].opt()])

            # ---- mean pool ----
            gp = gps.tile([128, G], f32, tag="gp")
            for t in range(TILES):
                nc.tensor.matmul(out=gp[:], lhsT=shb[:, t * F:(t + 1) * F],
                                 rhs=pools_t[:, t * G:(t + 1) * G],
                                 start=(t == 0), stop=(t == TILES - 1))
            gtile = tmpp.tile([128, G], f32, tag="gtile")
            nc.scalar.copy(out=gtile[:], in_=gp[:])
            nc.sync.dma_start(gt_in[:], gtile[:])
            nc.gpsimd.collective_compute(
                "AllReduce", mybir.AluOpType.add,
                replica_groups=[list(range(NCORES))],
                ins=[gt_in[:].opt()], outs=[gt_out[:].opt()])
            gfull = tmpp.tile([128, G], f32, tag="gfull")
            nc.sync.dma_start(gfull[:], gt_out[:])
            gt16 = tmpp.tile([128, G], f16, tag="gt16")
            nc.vector.tensor_copy(gt16[:], gfull[:])

            # ---- MLP (hidden dim sharded; partials reduce-scattered) ----
            mp = mps.tile([NHS, G], f32, tag="mp")
            nc.tensor.matmul(out=mp[:], lhsT=wm1s_t[:], rhs=gt16[:],
                             start=True, stop=True)
            mt16 = tmpp.tile([NHS, G], f16, tag="mt16")
            nc.scalar.activation(out=mt16[:], in_=mp[:], func=relu_fn,
                                 bias=bm1s_t[:])
            for gh in range(G // 128):
                op = mps.tile([128, NO], f32, tag="op")
                nc.tensor.matmul(
                    out=op[:], lhsT=mt16[:, 128 * gh:128 * (gh + 1)],
                    rhs=wm2s_t[:], start=True, stop=False)
                nc.tensor.matmul(
                    out=op[:], lhsT=ones1[:], rhs=bm2s8_t[:],
                    start=False, stop=True)
                ot = tmpp.tile([128, NO], f32, tag="ot")
                nc.scalar.copy(out=ot[:], in_=op[:])
                nc.sync.dma_start(part_d[128 * gh:128 * (gh + 1), :], ot[:])
            nc.gpsimd.collective_compute(
                "ReduceScatter", mybir.AluOpType.add,
                replica_groups=[list(range(NCORES))],
                ins=[part_d[:].opt()], outs=[rs_out[:].opt()])
            rt = tmpp.tile([GS, NO], f32, tag="rt")
            nc.sync.dma_start(rt[:], rs_out[:])
            nc.sync.dma_start(out[:], rt[:])

    nc.compile()
    return nc


def _get_built(inputs, variant="full", gcfg=None):
    import hashlib
    h = hashlib.sha1()
    h.update(np.ascontiguousarray(inputs["edge_index"]).tobytes())
    h.update(np.ascontiguousarray(inputs["batch"]).tobytes())
    key = (variant,
           tuple(sorted((k, v.shape, str(v.dtype)) for k, v in inputs.items())),
           h.hexdigest())
    if key not in _cache:
        geom, in_maps = _host_prep(**inputs)
        nc = _build_bass(geom, variant, gcfg)
        _cache[key] = (geom, nc)
    else:
        geom, nc = _cache[key]
        _, in_maps = _host_prep(**inputs)
    return geom, nc, in_maps


def kernel(**inputs):
    inputs = {k: np.asarray(v) for k, v in inputs.items()}
    geom, nc, in_maps = _get_built(inputs)
    from concourse.bass_utils import run_bass_kernel_spmd
    res = run_bass_kernel_spmd(nc, in_maps, list(range(NCORES)))
    return np.concatenate([np.asarray(res.results[c]["out"])
                           for c in range(NCORES)], axis=0)


# revision 3
# speedup vs baseline: 1.4103x; 1.1799x over previous
"""GCN encoder (3x GCNConv + mean-pool + MLP) as an 8-core Trainium2 Bass kernel.

v3: host-precomputed scatter matrices streamed from DRAM.

Sharding: nodes/edges partitioned by destination-node owner (8 shards).
Tables are W-premultiplied: tab0 = X@W0 (computed on device from per-core
transposed x shards, AllGathered), tab_{l+1} = relu(agg_l + b_l) @ W_{l+1}.
Per layer: per-edge source rows are gathered from the table (fp16 DRAM) with
dma_gather; the one-hot scatter matrices S (identical across layers, graph-
dependent only) are precomputed on the HOST in fp8e4m3 and streamed from DRAM
in groups, so the vector engine does no per-chunk work. Scatter-add happens
via PE matmul (lhsT = gathered messages fp16, rhs = S fp8). Self-loop and
mean-pool selection matrices are host-built fp16 and SBUF-resident. The psum
drain fuses bias+relu on the ACT engine in feature-major layout, and the
next-table matmul transposes to node-major for free. Final layer transposes
via PE for the mean-pool matmul; pooled sums are AllReduced; the MLP is
sharded over the hidden dim with a ReduceScatter of output partials, and each
core returns only its 32-graph slice of the output.
"""

import numpy as np

NCORES = 8
F = 128            # hidden width
G = 256            # number of graphs
NH = 512           # MLP hidden
NO = 256           # MLP out
CH = 128           # edges per chunk
BATCH_CH = 16      # chunks per dma_gather batch
WINW = 128         # dst nodes per PSUM accumulation window
SGRP = 8           # chunks per S-matrix stream DMA
WSPLIT = 32        # windows whose rows go in AllGather piece A
XT_FP8 = True      # ship x shards as fp8e4m3 (halves xt upload)

_cache = {}


def _host_prep(x, edge_index, batch, W0, b0, W1, b1, W2, b2, Wm1, bm1, Wm2, bm2):
    import ml_dtypes
    f8 = ml_dtypes.float8_e4m3

    N = x.shape[0]
    FI = x.shape[1]
    SH = -(-N // (NCORES * 128)) * 128      # shard size (nodes), 128-multiple
    NP = SH * NCORES
    TILES = SH // 128
    NWIN = -(-SH // WINW)
    RSPL = WSPLIT * WINW                    # shard rows in AllGather piece A
    LO = NCORES * RSPL                      # rows in table piece A (<= 32768)
    HI = NP - LO                            # rows in table piece B
    NHS = NH // NCORES                      # MLP hidden slice per core
    GS = G // NCORES                        # output graphs per core

    src = np.asarray(edge_index[0], dtype=np.int64)
    dst = np.asarray(edge_index[1], dtype=np.int64)
    deg = (np.bincount(np.concatenate([dst, np.arange(N, dtype=np.int64)]),
                       minlength=N).astype(np.float32))
    dis = np.where(deg > 0, 1.0 / np.sqrt(np.maximum(deg, 1.0)), 0.0).astype(np.float32)
    norm = dis[src] * dis[dst]

    # per-core edge selection, ordered by (window, class, dst).
    # class 0: src local-offset < RSPL (table piece A, gathered early);
    # class 1: src local-offset >= RSPL (table piece B).
    per_core = []
    for c in range(NCORES):
        base = c * SH
        sel = (dst >= base) & (dst < base + SH)
        es = src[sel].astype(np.int64)
        ed = (dst[sel] - base).astype(np.int64)
        en = norm[sel]
        sc, sl_ = es // SH, es % SH
        cl = (sl_ >= RSPL).astype(np.int64)
        ei = np.where(cl == 0, sc * RSPL + sl_, sc * (SH - RSPL) + (sl_ - RSPL))
        wi = ed // WINW
        order = np.lexsort((ed, cl, wi))
        per_core.append((ei[order], ed[order], en[order], cl[order], wi[order]))

    # chunk counts per (window, class), equalized across cores
    counts = np.zeros((NCORES, NWIN, 2), dtype=np.int64)
    for c in range(NCORES):
        _, _, _, cl, wi = per_core[c]
        for cls in (0, 1):
            counts[c, :, cls] = np.bincount(wi[cl == cls], minlength=NWIN)
    nch = -(-counts.max(axis=0) // CH)  # [NWIN, 2] chunks
    nch_cls = nch.sum(axis=0)          # total chunks per class
    ncht = int(nch.sum())

    # shared program schedule: windows -> list of (cls, cid)
    schedule = []
    cid_ctr = [0, 0]
    for w in range(NWIN):
        lst = []
        for cls in (0, 1):
            for _ in range(int(nch[w, cls])):
                lst.append((cls, cid_ctr[cls]))
                cid_ctr[cls] += 1
        schedule.append(lst)

    # per-core streams: compact idx [16, nch_cls*8] int16;
    # host-built scatter matrices s_all [128, ncht*WINW] fp8
    idx_streams = [[], []]
    s_alls = []
    for c in range(NCORES):
        es, ed, en, cl, wi = per_core[c]
        idx_parts = [[], []]
        s_all = np.zeros((128, ncht * WINW), dtype=np.float32)
        g = 0
        pos = 0
        for w in range(NWIN):
            for cls in (0, 1):
                n_e = int(counts[c, w, cls])
                tot = int(nch[w, cls]) * CH
                ge, gd, gn = es[pos:pos + n_e], ed[pos:pos + n_e], en[pos:pos + n_e]
                pos += n_e
                pad = tot - n_e
                iv = np.concatenate([ge, np.zeros(pad, np.int64)])
                dl = np.concatenate([gd - w * WINW, np.full(pad, -1, np.int64)])
                nr = np.concatenate([gn, np.zeros(pad, np.float32)])
                idx_parts[cls].append(iv.astype(np.int16))
                for k in range(tot // CH):
                    sl = slice(k * CH, (k + 1) * CH)
                    dlk, nrk = dl[sl], nr[sl]
                    valid = dlk >= 0
                    rows = np.nonzero(valid)[0]
                    s_all[rows, g * WINW + dlk[valid]] = nrk[valid]
                    g += 1
        assert g == ncht
        s_alls.append(s_all.astype(f8))
        for cls in (0, 1):
            arr = (np.concatenate(idx_parts[cls]) if idx_parts[cls]
                   else np.zeros(0, np.int16))
            assert arr.size == nch_cls[cls] * CH
            if arr.size:
                wrapped = arr.reshape(-1, 16).T       # [16, nch_cls*8]
            else:
                wrapped = np.zeros((16, 8), np.int16)  # dummy
            idx_streams[cls].append(np.ascontiguousarray(wrapped))

    # resident selection matrices: self-loop diag + mean-pool, per tile
    cnt = np.bincount(batch.astype(np.int64), minlength=G).astype(np.float32)
    invc_all = (1.0 / np.maximum(cnt, 1.0))[batch.astype(np.int64)]
    selfnr_all = dis * dis
    selfs_l, pools_l = [], []
    for c in range(NCORES):
        lo_n = c * SH
        hi_n = min((c + 1) * SH, N)
        nreal = max(0, hi_n - lo_n)
        selfs = np.zeros((128, TILES * 128), dtype=np.float16)
        pools = np.zeros((128, TILES * G), dtype=np.float16)
        for t in range(TILES):
            for p in range(128):
                v = t * 128 + p
                if v < nreal:
                    selfs[p, t * 128 + p] = selfnr_all[lo_n + v]
                    bgi = int(batch[lo_n + v])
                    pools[p, t * G + bgi] = invc_all[lo_n + v]
        selfs_l.append(selfs)
        pools_l.append(pools)

    # full MLP weights, graph-sharded tail: bm1f/bm2f wrapped per 128-block
    wm2f = np.zeros((128, (NH // 128) * (NO // 128) * 128), np.float16)
    for h in range(NH // 128):
        for oh in range(NO // 128):
            wm2f[:, (h * (NO // 128) + oh) * 128:
                 (h * (NO // 128) + oh + 1) * 128] = \
                Wm2[h * 128:(h + 1) * 128, oh * 128:(oh + 1) * 128]
    consts = {
        "w0": W0.astype(np.float16),                     # [FI, F]
        "w1": W1.astype(np.float16), "w2": W2.astype(np.float16),
        "bcols3": np.stack([b0, b1, b2], axis=1).astype(np.float32),  # [F, 3]
        "ident": np.eye(128, dtype=np.float16),
        "wm1f": np.ascontiguousarray(Wm1).astype(np.float16),   # [F, NH]
        "bm1f": np.ascontiguousarray(
            np.asarray(bm1).reshape(NH // 128, 128).T).astype(np.float32),
        "wm2f": wm2f,
        "bm2f": np.ascontiguousarray(
            np.asarray(bm2).reshape(NO // 128, 128).T).astype(np.float32),
    }
    xt_np = f8 if XT_FP8 else np.float16
    in_maps = []
    for c in range(NCORES):
        m = dict(consts)
        lo = c * SH
        hi = min((c + 1) * SH, N)
        xt = np.zeros((FI, SH), dtype=xt_np)
        xt[:, :hi - lo] = x[lo:hi].T.astype(xt_np)
        m["xt"] = np.ascontiguousarray(xt)
        m["idxlo"] = idx_streams[0][c]
        m["idxhi"] = idx_streams[1][c]
        m["s_all"] = s_alls[c]
        m["selfs"] = selfs_l[c]
        m["pools"] = pools_l[c]
        in_maps.append(m)

    geom = dict(N=N, FI=FI, NP=NP, SH=SH, TILES=TILES, NWIN=NWIN, LO=LO, HI=HI,
                NHS=NHS, GS=GS, nch=nch, nch_cls=[int(v) for v in nch_cls],
                ncht=ncht, schedule=schedule)
    return geom, in_maps


def _build_bass(geom, variant="full", gcfg=None):
    import concourse.bass as bass
    import concourse.tile as tile
    from concourse import bacc, mybir

    gcfg = dict(dict(batch=BATCH_CH, sp=False, nq=4, qg=True,
                     mbufs=8, sbufs=6, wbufs=3, prep=False),
                **(gcfg or {}))
    BCH = gcfg["batch"]

    f16, f32, i16 = mybir.dt.float16, mybir.dt.float32, mybir.dt.int16
    f8 = mybir.dt.float8e4
    fxt = f8 if XT_FP8 else f16
    FI, NP, SH, TILES, NWIN = (geom["FI"], geom["NP"], geom["SH"],
                               geom["TILES"], geom["NWIN"])
    LO, HI, NHS, GS = geom["LO"], geom["HI"], geom["NHS"], geom["GS"]
    nch, nch_cls, ncht = geom["nch"], geom["nch_cls"], geom["ncht"]
    schedule = geom["schedule"]
    NSG = -(-ncht // SGRP)  # number of S stream groups

    nc = bacc.Bacc("TRN2", target_bir_lowering=False, debug=False,
                   num_devices=NCORES, num_swdge_queues=gcfg["nq"])

    xt = nc.dram_tensor("xt", [FI, SH], fxt, kind="ExternalInput")
    idxlo = nc.dram_tensor("idxlo", [16, max(nch_cls[0] * 8, 8)], i16,
                           kind="ExternalInput")
    idxhi = nc.dram_tensor("idxhi", [16, max(nch_cls[1] * 8, 8)], i16,
                           kind="ExternalInput")
    s_all = nc.dram_tensor("s_all", [128, ncht * WINW], f8, kind="ExternalInput")
    selfs = nc.dram_tensor("selfs", [128, TILES * 128], f16, kind="ExternalInput")
    pools = nc.dram_tensor("pools", [128, TILES * G], f16, kind="ExternalInput")
    w0 = nc.dram_tensor("w0", [FI, F], f16, kind="ExternalInput")
    w1 = nc.dram_tensor("w1", [F, F], f16, kind="ExternalInput")
    w2 = nc.dram_tensor("w2", [F, F], f16, kind="ExternalInput")
    bcols3 = nc.dram_tensor("bcols3", [F, 3], f32, kind="ExternalInput")
    ident_d = nc.dram_tensor("ident", [128, 128], f16, kind="ExternalInput")
    wm1f = nc.dram_tensor("wm1f", [F, NH], f16, kind="ExternalInput")
    bm1f = nc.dram_tensor("bm1f", [128, NH // 128], f32, kind="ExternalInput")
    wm2f = nc.dram_tensor("wm2f", [128, (NH // 128) * (NO // 128) * 128], f16,
                          kind="ExternalInput")
    bm2f = nc.dram_tensor("bm2f", [128, NO // 128], f32, kind="ExternalInput")
    out = nc.dram_tensor("out", [NO, GS], f32, kind="ExternalOutput")

    shard_d = nc.dram_tensor("shard_d", [SH, F], f16)
    RSPL = LO // NCORES
    tabsA = [nc.dram_tensor(f"tabA{l}", [LO, F], f16, addr_space="Shared")
             for l in range(3)]
    tabsB = [nc.dram_tensor(f"tabB{l}", [HI, F], f16, addr_space="Shared")
             for l in range(3)]
    gt_in = nc.dram_tensor("gt_in", [G, 128], f32)
    rs_gt = nc.dram_tensor("rs_gt", [GS, 128], f32)

    shb = nc.alloc_sbuf_tensor("shb", [128, TILES * F], f16)

    relu_fn = mybir.ActivationFunctionType.Relu

    with tile.TileContext(nc) as tc:
        with (
            tc.tile_pool(name="res", bufs=1) as res,
            tc.tile_pool(name="msg", bufs=gcfg["mbufs"]) as msgp,
            tc.tile_pool(name="sst", bufs=gcfg["sbufs"]) as sstp,
            tc.tile_pool(name="agg", bufs=2) as aggp,
            tc.tile_pool(name="tmp", bufs=2) as tmpp,
            tc.tile_pool(name="wps", bufs=gcfg["wbufs"], space="PSUM") as wps,
            tc.tile_pool(name="hps", bufs=2, space="PSUM") as hps,
            tc.tile_pool(name="gps", bufs=2, space="PSUM") as gps,
        ):
            # ---- resident loads ----
            def load(t_dram, shape, dtype):
                t = res.tile(shape, dtype, tag=t_dram.name)
                nc.sync.dma_start(t[:], t_dram[:])
                return t

            idx_t = []
            for cls, t_dram in ((0, idxlo), (1, idxhi)):
                w = max(nch_cls[cls] * 8, 8)
                t = res.tile([128, w], i16, tag=f"idx{cls}", name=f"idxt{cls}")
                for k in range(8):
                    nc.sync.dma_start(t[16 * k:16 * (k + 1), :], t_dram[:])
                idx_t.append(t)
            xt_t = load(xt, [FI, SH], fxt)
            selfs_t = load(selfs, [128, TILES * 128], f16)
            pools_t = load(pools, [128, TILES * G], f16)
            w0_t = load(w0, [FI, F], f16)
            w_t = {1: load(w1, [F, F], f16), 2: load(w2, [F, F], f16)}
            bcols3_t = load(bcols3, [F, 3], f32)
            ident = load(ident_d, [128, 128], f16)
            wm1f_t = load(wm1f, [F, NH], f16)
            bm1f_t = load(bm1f, [128, NH // 128], f32)
            wm2f_t = load(wm2f, [128, (NH // 128) * (NO // 128) * 128], f16)
            bm2f_t = load(bm2f, [128, NO // 128], f32)
            dma_sems = [nc.alloc_semaphore(f"gsem{q}")
                        for q in range(gcfg["nq"])] if gcfg["prep"] else None

            # ---- T0 = X @ W0 (per-shard), node-major into shb ----
            WSPLIT = LO // (NCORES * WINW)  # windows in AllGather piece A

            def shard_write(lo_t, hi_t):
                nc.sync.dma_start(
                    shard_d[lo_t * 128:hi_t * 128, :]
                    .rearrange("(t p) f -> p t f", p=128),
                    shb[:, lo_t * F:hi_t * F]
                    .rearrange("p (t f) -> p t f", f=F))

            def allgather_piece(l_next, piece):
                if piece == 0:
                    ins_ap, outs_ap = shard_d[0:RSPL, :], tabsA[l_next][:]
                else:
                    ins_ap, outs_ap = shard_d[RSPL:SH, :], tabsB[l_next][:]
                nc.gpsimd.collective_compute(
                    "AllGather", mybir.AluOpType.bypass,
                    replica_groups=[list(range(NCORES))],
                    ins=[ins_ap.opt()], outs=[outs_ap.opt()])

            for t in range(TILES):
                t0p = hps.tile([128, F], f32, tag="hp")
                nc.tensor.matmul(out=t0p[:], lhsT=xt_t[:, 128 * t:128 * (t + 1)],
                                 rhs=w0_t[:], start=True, stop=True)
                nc.scalar.copy(out=shb[:, t * F:(t + 1) * F], in_=t0p[:])
                if t == WSPLIT - 1:
                    shard_write(0, WSPLIT)
                    allgather_piece(0, 0)
            shard_write(WSPLIT, TILES)
            allgather_piece(0, 1)

            # ---- 3 GCN layers ----
            gctr = [0]  # global gather counter for queue round-robin
            for l in range(3):
                tbl_ap = [tabsA[l][:], tabsB[l][:]]
                issued = [-1, -1]
                cur = [None, None]
                sg_cur = [-1]
                st_cur = [None]
                g = 0
                for w in range(NWIN):
                    width = min(WINW, SH - w * WINW)
                    chunks = schedule[w]
                    ops = [("c", x) for x in chunks]
                    selfop = ("s", 0)
                    ops = (ops[:1] + [selfop] + ops[1:]) if ops else [selfop]
                    ps = wps.tile([128, WINW], f32, tag="wps")
                    no_chunks = not chunks
                    for j, op in enumerate(ops):
                        first, last = (j == 0), (j == len(ops) - 1)
                        if op[0] == "s":
                            t_idx = w
                            nc.tensor.matmul(
                                out=ps[:, :width],
                                lhsT=shb[:, t_idx * F:(t_idx + 1) * F],
                                rhs=selfs_t[:, t_idx * 128:t_idx * 128 + width],
                                start=(True if no_chunks else False),
                                stop=last, skip_group_check=True)
                            continue
                        cls, cid = op[1]
                        b, slab = divmod(cid, BCH)
                        if b != issued[cls]:
                            nb = min(BCH, nch_cls[cls] - b * BCH)
                            mt = msgp.tile([128, BCH, F], f16, tag=f"msg{cls}")
                            if variant == "memset":
                                nc.vector.memset(mt[:, :nb, :], 0.0)
                            else:
                                qn = ((gctr[0] if gcfg["qg"] else b)
                                      % gcfg["nq"])
                                gctr[0] += 1
                                if gcfg["prep"]:
                                    nc.gpsimd.dma_gather(
                                        mt[:, :nb, :], tbl_ap[cls],
                                        idx_t[cls][:, b * (BCH * 8):
                                                   b * (BCH * 8) + nb * 8],
                                        nb * CH, nb * CH, F,
                                        single_packet=gcfg["sp"],
                                        prepare_only=True,
                                        sem=dma_sems[qn],
                                        queue_num=qn)
                                    nc.gpsimd.trigger_dma(count=None,
                                                          queue_num=qn)
                                else:
                                    nc.gpsimd.dma_gather(
                                        mt[:, :nb, :], tbl_ap[cls],
                                        idx_t[cls][:, b * (BCH * 8):
                                                   b * (BCH * 8) + nb * 8],
                                        nb * CH, nb * CH, F,
                                        single_packet=gcfg["sp"],
                                        queue_num=qn)
                            issued[cls] = b
                            cur[cls] = mt
                        sg = g // SGRP
                        if sg != sg_cur[0]:
                            ngc = min(SGRP, ncht - sg * SGRP)
                            st = sstp.tile([128, SGRP * WINW], f8, tag="sst")
                            nc.sync.dma_start(
                                st[:, :ngc * WINW],
                                s_all[:, sg * SGRP * WINW:
                                      (sg * SGRP + ngc) * WINW])
                            sg_cur[0] = sg
                            st_cur[0] = st
                        so = (g - sg * SGRP) * WINW
                        nc.tensor.matmul(
                            out=ps[:, :width], lhsT=cur[cls][:, slab, :],
                            rhs=st_cur[0][:, so:so + width],
                            start=first, stop=last)
                        g += 1
                    # drain window (width <= 128): fuse bias (+relu) on ACT
                    t_idx = w
                    dst_sl = shb[:, t_idx * F:(t_idx + 1) * F]
                    if l < 2:
                        aggT = aggp.tile([128, 128], f16, tag="aggT")
                        nc.scalar.activation(
                            out=aggT[:, :width], in_=ps[:, :width],
                            func=relu_fn, bias=bcols3_t[:, l:l + 1])
                        tp = hps.tile([128, F], f32, tag="hp")
                        nc.tensor.matmul(out=tp[:], lhsT=aggT[:],
                                         rhs=w_t[l + 1][:],
                                         start=True, stop=True)
                        nc.scalar.copy(out=dst_sl, in_=tp[:])
                    else:
                        aggT = aggp.tile([128, 128], f16, tag="aggT")
                        nc.scalar.add(out=aggT[:, :width], in_=ps[:, :width],
                                      add=bcols3_t[:, 2:3])
                        tp = hps.tile([128, 128], f16, tag="hp",
                                      name="tppose")
                        nc.tensor.transpose(out=tp[:], in_=aggT[:],
                                            identity=ident[:])
                        nc.scalar.copy(out=dst_sl, in_=tp[:])
                    if l < 2 and w == WSPLIT - 1:
                        shard_write(0, WSPLIT)
                        allgather_piece(l + 1, 0)
                assert g == ncht
                if l < 2:
                    shard_write(WSPLIT, TILES)
                    allgather_piece(l + 1, 1)

            # ---- mean pool, graph-major [G, F], reduce-scattered over graphs
            gpp = [gps.tile([128, F], f32, tag="gp", name=f"gp{gh}")
                   for gh in range(G // 128)]
            for t in range(TILES):
                for gh in range(G // 128):
                    nc.tensor.matmul(
                        out=gpp[gh][:],
                        lhsT=pools_t[:, t * G + gh * 128:t * G + gh * 128 + 128],
                        rhs=shb[:, t * F:(t + 1) * F],
                        start=(t == 0), stop=(t == TILES - 1))
            for gh in range(G // 128):
                gt = tmpp.tile([128, F], f32, tag="gtile")
                nc.scalar.copy(out=gt[:], in_=gpp[gh][:])
                nc.sync.dma_start(gt_in[gh * 128:(gh + 1) * 128, :], gt[:])
            nc.gpsimd.collective_compute(
                "ReduceScatter", mybir.AluOpType.add,
                replica_groups=[list(range(NCORES))],
                ins=[gt_in[:].opt()], outs=[rs_gt[:].opt()])

            # ---- MLP on this core's GS graphs ----
            rst = tmpp.tile([GS, 128], f32, tag="rst")
            nc.sync.dma_start(rst[:], rs_gt[:])
            rst16 = tmpp.tile([GS, 128], f16, tag="rst16")
            nc.vector.tensor_copy(rst16[:], rst[:])
            tpg = hps.tile([128, GS], f16, tag="hp", name="tpgt")
            nc.tensor.transpose(out=tpg[:], in_=rst16[:],
                                identity=ident[0:GS, 0:GS])
            gt16s = tmpp.tile([128, GS], f16, tag="gt16s")
            nc.scalar.copy(out=gt16s[:], in_=tpg[:])
            m16 = tmpp.tile([128, (NH // 128) * GS], f16, tag="m16")
            for h in range(NH // 128):
                m1p = gps.tile([128, GS], f32, tag="gp", name=f"m1p{h}")
                nc.tensor.matmul(out=m1p[:], lhsT=wm1f_t[:, h * 128:(h + 1) * 128],
                                 rhs=gt16s[:], start=True, stop=True)
                nc.scalar.activation(out=m16[:, h * GS:(h + 1) * GS], in_=m1p[:],
                                     func=relu_fn, bias=bm1f_t[:, h:h + 1])
            for oh in range(NO // 128):
                m2p = gps.tile([128, GS], f32, tag="gp", name=f"m2p{oh}")
                for h in range(NH // 128):
                    blk = (h * (NO // 128) + oh) * 128
                    nc.tensor.matmul(out=m2p[:],
                                     lhsT=wm2f_t[:, blk:blk + 128],
                                     rhs=m16[:, h * GS:(h + 1) * GS],
                                     start=(h == 0), stop=(h == NH // 128 - 1))
                ot = tmpp.tile([128, GS], f32, tag="ot")
                nc.scalar.add(out=ot[:], in_=m2p[:], add=bm2f_t[:, oh:oh + 1])
                nc.sync.dma_start(out[oh * 128:(oh + 1) * 128, :], ot[:])

    nc.compile()
    return nc


def _get_built(inputs, variant="full", gcfg=None):
    import hashlib
    h = hashlib.sha1()
    h.update(np.ascontiguousarray(inputs["edge_index"]).tobytes())
    h.update(np.ascontiguousarray(inputs["batch"]).tobytes())
    key = (variant,
           tuple(sorted((k, v.shape, str(v.dtype)) for k, v in inputs.items())),
           h.hexdigest())
    if key not in _cache:
        geom, in_maps = _host_prep(**inputs)
        nc = _build_bass(geom, variant, gcfg)
        _cache[key] = (geom, nc)
    else:
        geom, nc = _cache[key]
        _, in_maps = _host_prep(**inputs)
    return geom, nc, in_maps


def kernel(**inputs):
    inputs = {k: np.asarray(v) for k, v in inputs.items()}
    geom, nc, in_maps = _get_built(inputs)
    from concourse.bass_utils import run_bass_kernel_spmd
    res = run_bass_kernel_spmd(nc, in_maps, list(range(NCORES)))
    # per-core output is [NO, GS] (transposed); assemble to [G, NO]
    return np.concatenate([np.asarray(res.results[c]["out"]).T
                           for c in range(NCORES)], axis=0)


# revision 4
# speedup vs baseline: 1.4441x; 1.0240x over previous
"""GCN encoder (3x GCNConv + mean-pool + MLP) as an 8-core Trainium2 Bass kernel.

v3: host-precomputed scatter matrices streamed from DRAM.

Sharding: nodes/edges partitioned by destination-node owner (8 shards).
Tables are W-premultiplied: tab0 = X@W0 (computed on device from per-core
transposed x shards, AllGathered), tab_{l+1} = relu(agg_l + b_l) @ W_{l+1}.
Per layer: per-edge source rows are gathered from the table (fp16 DRAM) with
dma_gather; the one-hot scatter matrices S (identical across layers, graph-
dependent only) are precomputed on the HOST in fp8e4m3 and streamed from DRAM
in groups, so the vector engine does no per-chunk work. Scatter-add happens
via PE matmul (lhsT = gathered messages fp16, rhs = S fp8). Self-loop and
mean-pool selection matrices are host-built fp16 and SBUF-resident. The psum
drain fuses bias+relu on the ACT engine in feature-major layout, and the
next-table matmul transposes to node-major for free. Final layer transposes
via PE for the mean-pool matmul; pooled sums are AllReduced; the MLP is
sharded over the hidden dim with a ReduceScatter of output partials, and each
core returns only its 32-graph slice of the output.
"""

import numpy as np

NCORES = 8
F = 128            # hidden width
G = 256            # number of graphs
NH = 512           # MLP hidden
NO = 256           # MLP out
CH = 128           # edges per chunk
BATCH_CH = 16      # chunks per dma_gather batch
WINW = 256         # dst nodes per PSUM accumulation window
SGRP = 8           # chunks per S-matrix stream DMA
WSPLIT = 16        # windows whose rows go in AllGather piece A
XT_FP8 = True      # ship x shards as fp8e4m3 (halves xt upload)

_cache = {}


def _host_prep(x, edge_index, batch, W0, b0, W1, b1, W2, b2, Wm1, bm1, Wm2, bm2):
    import ml_dtypes
    f8 = ml_dtypes.float8_e4m3

    N = x.shape[0]
    FI = x.shape[1]
    SH = -(-N // (NCORES * 128)) * 128      # shard size (nodes), 128-multiple
    NP = SH * NCORES
    TILES = SH // 128
    NWIN = -(-SH // WINW)
    RSPL = WSPLIT * WINW                    # shard rows in AllGather piece A
    LO = NCORES * RSPL                      # rows in table piece A (<= 32768)
    HI = NP - LO                            # rows in table piece B
    NHS = NH // NCORES                      # MLP hidden slice per core
    GS = G // NCORES                        # output graphs per core

    src = np.asarray(edge_index[0], dtype=np.int64)
    dst = np.asarray(edge_index[1], dtype=np.int64)
    deg = (np.bincount(np.concatenate([dst, np.arange(N, dtype=np.int64)]),
                       minlength=N).astype(np.float32))
    dis = np.where(deg > 0, 1.0 / np.sqrt(np.maximum(deg, 1.0)), 0.0).astype(np.float32)
    norm = dis[src] * dis[dst]

    # per-core edge selection, ordered by (window, class, dst).
    # class 0: src local-offset < RSPL (table piece A, gathered early);
    # class 1: src local-offset >= RSPL (table piece B).
    per_core = []
    for c in range(NCORES):
        base = c * SH
        sel = (dst >= base) & (dst < base + SH)
        es = src[sel].astype(np.int64)
        ed = (dst[sel] - base).astype(np.int64)
        en = norm[sel]
        sc, sl_ = es // SH, es % SH
        cl = (sl_ >= RSPL).astype(np.int64)
        ei = np.where(cl == 0, sc * RSPL + sl_, sc * (SH - RSPL) + (sl_ - RSPL))
        wi = ed // WINW
        order = np.lexsort((ed, cl, wi))
        per_core.append((ei[order], ed[order], en[order], cl[order], wi[order]))

    # chunk counts per (window, class), equalized across cores
    counts = np.zeros((NCORES, NWIN, 2), dtype=np.int64)
    for c in range(NCORES):
        _, _, _, cl, wi = per_core[c]
        for cls in (0, 1):
            counts[c, :, cls] = np.bincount(wi[cl == cls], minlength=NWIN)
    nch = -(-counts.max(axis=0) // CH)  # [NWIN, 2] chunks
    nch_cls = nch.sum(axis=0)          # total chunks per class
    ncht = int(nch.sum())

    # shared program schedule: windows -> list of (cls, cid)
    schedule = []
    cid_ctr = [0, 0]
    for w in range(NWIN):
        lst = []
        for cls in (0, 1):
            for _ in range(int(nch[w, cls])):
                lst.append((cls, cid_ctr[cls]))
                cid_ctr[cls] += 1
        schedule.append(lst)

    # per-core streams: compact idx [16, nch_cls*8] int16;
    # host-built scatter matrices s_all [128, ncht*WINW] fp8
    idx_streams = [[], []]
    s_alls = []
    for c in range(NCORES):
        es, ed, en, cl, wi = per_core[c]
        idx_parts = [[], []]
        s_all = np.zeros((128, ncht * WINW), dtype=np.float32)
        g = 0
        pos = 0
        for w in range(NWIN):
            for cls in (0, 1):
                n_e = int(counts[c, w, cls])
                tot = int(nch[w, cls]) * CH
                ge, gd, gn = es[pos:pos + n_e], ed[pos:pos + n_e], en[pos:pos + n_e]
                pos += n_e
                pad = tot - n_e
                iv = np.concatenate([ge, np.zeros(pad, np.int64)])
                dl = np.concatenate([gd - w * WINW, np.full(pad, -1, np.int64)])
                nr = np.concatenate([gn, np.zeros(pad, np.float32)])
                idx_parts[cls].append(iv.astype(np.int16))
                for k in range(tot // CH):
                    sl = slice(k * CH, (k + 1) * CH)
                    dlk, nrk = dl[sl], nr[sl]
                    valid = dlk >= 0
                    rows = np.nonzero(valid)[0]
                    s_all[rows, g * WINW + dlk[valid]] = nrk[valid]
                    g += 1
        assert g == ncht
        s_alls.append(s_all.astype(f8))
        for cls in (0, 1):
            arr = (np.concatenate(idx_parts[cls]) if idx_parts[cls]
                   else np.zeros(0, np.int16))
            assert arr.size == nch_cls[cls] * CH
            if arr.size:
                wrapped = arr.reshape(-1, 16).T       # [16, nch_cls*8]
            else:
                wrapped = np.zeros((16, 8), np.int16)  # dummy
            idx_streams[cls].append(np.ascontiguousarray(wrapped))

    # resident selection matrices: self-loop diag + mean-pool, per tile
    cnt = np.bincount(batch.astype(np.int64), minlength=G).astype(np.float32)
    invc_all = (1.0 / np.maximum(cnt, 1.0))[batch.astype(np.int64)]
    selfnr_all = dis * dis
    selfs_l, pools_l = [], []
    for c in range(NCORES):
        lo_n = c * SH
        hi_n = min((c + 1) * SH, N)
        nreal = max(0, hi_n - lo_n)
        selfs = np.zeros((128, TILES * 128), dtype=np.float16)
        pools = np.zeros((128, TILES * G), dtype=np.float16)
        for t in range(TILES):
            for p in range(128):
                v = t * 128 + p
                if v < nreal:
                    selfs[p, t * 128 + p] = selfnr_all[lo_n + v]
                    bgi = int(batch[lo_n + v])
                    pools[p, t * G + bgi] = invc_all[lo_n + v]
        selfs_l.append(selfs)
        pools_l.append(pools)

    # full MLP weights, graph-sharded tail: bm1f/bm2f wrapped per 128-block
    wm2f = np.zeros((128, (NH // 128) * (NO // 128) * 128), np.float16)
    for h in range(NH // 128):
        for oh in range(NO // 128):
            wm2f[:, (h * (NO // 128) + oh) * 128:
                 (h * (NO // 128) + oh + 1) * 128] = \
                Wm2[h * 128:(h + 1) * 128, oh * 128:(oh + 1) * 128]
    consts = {
        "w0": W0.astype(np.float16),                     # [FI, F]
        "w1": W1.astype(np.float16), "w2": W2.astype(np.float16),
        "bcols3": np.stack([b0, b1, b2], axis=1).astype(np.float32),  # [F, 3]
        "ident": np.eye(128, dtype=np.float16),
        "wm1f": np.ascontiguousarray(Wm1).astype(np.float16),   # [F, NH]
        "bm1f": np.ascontiguousarray(
            np.asarray(bm1).reshape(NH // 128, 128).T).astype(np.float32),
        "wm2f": wm2f,
        "bm2f": np.ascontiguousarray(
            np.asarray(bm2).reshape(NO // 128, 128).T).astype(np.float32),
    }
    xt_np = f8 if XT_FP8 else np.float16
    in_maps = []
    for c in range(NCORES):
        m = dict(consts)
        lo = c * SH
        hi = min((c + 1) * SH, N)
        xt = np.zeros((FI, SH), dtype=xt_np)
        xt[:, :hi - lo] = x[lo:hi].T.astype(xt_np)
        m["xt"] = np.ascontiguousarray(xt)
        m["idxlo"] = idx_streams[0][c]
        m["idxhi"] = idx_streams[1][c]
        m["s_all"] = s_alls[c]
        m["selfs"] = selfs_l[c]
        m["pools"] = pools_l[c]
        in_maps.append(m)

    geom = dict(N=N, FI=FI, NP=NP, SH=SH, TILES=TILES, NWIN=NWIN, LO=LO, HI=HI,
                NHS=NHS, GS=GS, nch=nch, nch_cls=[int(v) for v in nch_cls],
                ncht=ncht, schedule=schedule)
    return geom, in_maps


def _build_bass(geom, variant="full", gcfg=None):
    import concourse.bass as bass
    import concourse.tile as tile
    from concourse import bacc, mybir

    gcfg = dict(dict(batch=BATCH_CH, sp=False, nq=4, qg=True,
                     mbufs=8, sbufs=6, wbufs=3, prep=False, pref=0),
                **(gcfg or {}))
    BCH = gcfg["batch"]

    f16, f32, i16 = mybir.dt.float16, mybir.dt.float32, mybir.dt.int16
    f8 = mybir.dt.float8e4
    fxt = f8 if XT_FP8 else f16
    FI, NP, SH, TILES, NWIN = (geom["FI"], geom["NP"], geom["SH"],
                               geom["TILES"], geom["NWIN"])
    LO, HI, NHS, GS = geom["LO"], geom["HI"], geom["NHS"], geom["GS"]
    nch, nch_cls, ncht = geom["nch"], geom["nch_cls"], geom["ncht"]
    schedule = geom["schedule"]
    NSG = -(-ncht // SGRP)  # number of S stream groups

    nc = bacc.Bacc("TRN2", target_bir_lowering=False, debug=False,
                   num_devices=NCORES, num_swdge_queues=gcfg["nq"])

    xt = nc.dram_tensor("xt", [FI, SH], fxt, kind="ExternalInput")
    idxlo = nc.dram_tensor("idxlo", [16, max(nch_cls[0] * 8, 8)], i16,
                           kind="ExternalInput")
    idxhi = nc.dram_tensor("idxhi", [16, max(nch_cls[1] * 8, 8)], i16,
                           kind="ExternalInput")
    s_all = nc.dram_tensor("s_all", [128, ncht * WINW], f8, kind="ExternalInput")
    selfs = nc.dram_tensor("selfs", [128, TILES * 128], f16, kind="ExternalInput")
    pools = nc.dram_tensor("pools", [128, TILES * G], f16, kind="ExternalInput")
    w0 = nc.dram_tensor("w0", [FI, F], f16, kind="ExternalInput")
    w1 = nc.dram_tensor("w1", [F, F], f16, kind="ExternalInput")
    w2 = nc.dram_tensor("w2", [F, F], f16, kind="ExternalInput")
    bcols3 = nc.dram_tensor("bcols3", [F, 3], f32, kind="ExternalInput")
    ident_d = nc.dram_tensor("ident", [128, 128], f16, kind="ExternalInput")
    wm1f = nc.dram_tensor("wm1f", [F, NH], f16, kind="ExternalInput")
    bm1f = nc.dram_tensor("bm1f", [128, NH // 128], f32, kind="ExternalInput")
    wm2f = nc.dram_tensor("wm2f", [128, (NH // 128) * (NO // 128) * 128], f16,
                          kind="ExternalInput")
    bm2f = nc.dram_tensor("bm2f", [128, NO // 128], f32, kind="ExternalInput")
    out = nc.dram_tensor("out", [NO, GS], f32, kind="ExternalOutput")

    shard_d = nc.dram_tensor("shard_d", [SH, F], f16)
    RSPL = LO // NCORES
    tabsA = [nc.dram_tensor(f"tabA{l}", [LO, F], f16, addr_space="Shared")
             for l in range(3)]
    tabsB = [nc.dram_tensor(f"tabB{l}", [HI, F], f16, addr_space="Shared")
             for l in range(3)]
    gt_in = nc.dram_tensor("gt_in", [G, 128], f32)
    rs_gt = nc.dram_tensor("rs_gt", [GS, 128], f32)

    shb = nc.alloc_sbuf_tensor("shb", [128, TILES * F], f16)

    relu_fn = mybir.ActivationFunctionType.Relu

    with tile.TileContext(nc) as tc:
        with (
            tc.tile_pool(name="res", bufs=1) as res,
            tc.tile_pool(name="msg", bufs=gcfg["mbufs"]) as msgp,
            tc.tile_pool(name="sst", bufs=gcfg["sbufs"]) as sstp,
            tc.tile_pool(name="agg", bufs=2) as aggp,
            tc.tile_pool(name="tmp", bufs=2) as tmpp,
            tc.tile_pool(name="wps", bufs=gcfg["wbufs"], space="PSUM") as wps,
            tc.tile_pool(name="hps", bufs=2, space="PSUM") as hps,
            tc.tile_pool(name="gps", bufs=2, space="PSUM") as gps,
        ):
            # ---- resident loads ----
            def load(t_dram, shape, dtype):
                t = res.tile(shape, dtype, tag=t_dram.name)
                nc.sync.dma_start(t[:], t_dram[:])
                return t

            idx_t = []
            for cls, t_dram in ((0, idxlo), (1, idxhi)):
                w = max(nch_cls[cls] * 8, 8)
                t = res.tile([128, w], i16, tag=f"idx{cls}", name=f"idxt{cls}")
                for k in range(8):
                    nc.sync.dma_start(t[16 * k:16 * (k + 1), :], t_dram[:])
                idx_t.append(t)
            xt_t = load(xt, [FI, SH], fxt)
            selfs_t = load(selfs, [128, TILES * 128], f16)
            pools_t = load(pools, [128, TILES * G], f16)
            w0_t = load(w0, [FI, F], f16)
            w_t = {1: load(w1, [F, F], f16), 2: load(w2, [F, F], f16)}
            bcols3_t = load(bcols3, [F, 3], f32)
            ident = load(ident_d, [128, 128], f16)
            wm1f_t = load(wm1f, [F, NH], f16)
            bm1f_t = load(bm1f, [128, NH // 128], f32)
            wm2f_t = load(wm2f, [128, (NH // 128) * (NO // 128) * 128], f16)
            bm2f_t = load(bm2f, [128, NO // 128], f32)
            dma_sems = [nc.alloc_semaphore(f"gsem{q}")
                        for q in range(gcfg["nq"])] if gcfg["prep"] else None

            # ---- T0 = X @ W0 (per-shard), node-major into shb ----
            WSPLIT = LO // (NCORES * WINW)  # windows in AllGather piece A
            WPT = WINW // 128

            def shard_write(lo_t, hi_t):
                nc.sync.dma_start(
                    shard_d[lo_t * 128:hi_t * 128, :]
                    .rearrange("(t p) f -> p t f", p=128),
                    shb[:, lo_t * F:hi_t * F]
                    .rearrange("p (t f) -> p t f", f=F))

            def allgather_piece(l_next, piece):
                if piece == 0:
                    ins_ap, outs_ap = shard_d[0:RSPL, :], tabsA[l_next][:]
                else:
                    ins_ap, outs_ap = shard_d[RSPL:SH, :], tabsB[l_next][:]
                nc.gpsimd.collective_compute(
                    "AllGather", mybir.AluOpType.bypass,
                    replica_groups=[list(range(NCORES))],
                    ins=[ins_ap.opt()], outs=[outs_ap.opt()])

            for t in range(TILES):
                t0p = hps.tile([128, F], f32, tag="hp")
                nc.tensor.matmul(out=t0p[:], lhsT=xt_t[:, 128 * t:128 * (t + 1)],
                                 rhs=w0_t[:], start=True, stop=True)
                nc.scalar.copy(out=shb[:, t * F:(t + 1) * F], in_=t0p[:])
                if t == WSPLIT * WPT - 1:
                    shard_write(0, WSPLIT * WPT)
                    allgather_piece(0, 0)
            shard_write(WSPLIT * WPT, TILES)
            allgather_piece(0, 1)

            # ---- 3 GCN layers ----
            gctr = [0]  # global gather counter for queue round-robin
            prefetched = [{}, {}]  # cls -> {batch: msg tile}, for next layer

            def issue_batch(cls, b, tbl_ap_l):
                nb = min(BCH, nch_cls[cls] - b * BCH)
                mt = msgp.tile([128, BCH, F], f16, tag=f"msg{cls}")
                qn = (gctr[0] if gcfg["qg"] else b) % gcfg["nq"]
                gctr[0] += 1
                nc.gpsimd.dma_gather(
                    mt[:, :nb, :], tbl_ap_l[cls],
                    idx_t[cls][:, b * (BCH * 8):b * (BCH * 8) + nb * 8],
                    nb * CH, nb * CH, F,
                    single_packet=gcfg["sp"], queue_num=qn)
                return mt

            for l in range(3):
                tbl_ap = [tabsA[l][:], tabsB[l][:]]
                issued = [-1, -1]
                cur = [None, None]
                sg_cur = [-1]
                st_cur = [None]
                g = 0
                WPT = WINW // 128
                for w in range(NWIN):
                    width = min(WINW, SH - w * WINW)
                    chunks = schedule[w]
                    ops = [("c", x) for x in chunks]
                    selfops = [("s", sub) for sub in range(width // 128)]
                    ops = (ops[:1] + selfops + ops[1:]) if ops else selfops
                    ps = wps.tile([128, WINW], f32, tag="wps")
                    no_chunks = not chunks
                    for j, op in enumerate(ops):
                        first, last = (j == 0), (j == len(ops) - 1)
                        if op[0] == "s":
                            sub = op[1]
                            t_idx = w * WPT + sub
                            nc.tensor.matmul(
                                out=ps[:, sub * 128:sub * 128 + 128],
                                lhsT=shb[:, t_idx * F:(t_idx + 1) * F],
                                rhs=selfs_t[:, t_idx * 128:(t_idx + 1) * 128],
                                start=(True if no_chunks else False),
                                stop=last, skip_group_check=True)
                            continue
                        cls, cid = op[1]
                        b, slab = divmod(cid, BCH)
                        if b != issued[cls]:
                            if b in prefetched[cls]:
                                mt = prefetched[cls].pop(b)
                            else:
                                mt = issue_batch(cls, b, tbl_ap)
                            issued[cls] = b
                            cur[cls] = mt
                        sg = g // SGRP
                        if sg != sg_cur[0]:
                            ngc = min(SGRP, ncht - sg * SGRP)
                            st = sstp.tile([128, SGRP * WINW], f8, tag="sst")
                            nc.sync.dma_start(
                                st[:, :ngc * WINW],
                                s_all[:, sg * SGRP * WINW:
                                      (sg * SGRP + ngc) * WINW])
                            sg_cur[0] = sg
                            st_cur[0] = st
                        so = (g - sg * SGRP) * WINW
                        nc.tensor.matmul(
                            out=ps[:, :width], lhsT=cur[cls][:, slab, :],
                            rhs=st_cur[0][:, so:so + width],
                            start=first, stop=last)
                        g += 1
                    # drain window: fuse bias (+relu) on ACT per subtile
                    for sub in range(width // 128):
                        t_idx = w * WPT + sub
                        pslice = ps[:, sub * 128:(sub + 1) * 128]
                        dst_sl = shb[:, t_idx * F:(t_idx + 1) * F]
                        if l < 2:
                            aggT = aggp.tile([128, 128], f16, tag="aggT")
                            nc.scalar.activation(
                                out=aggT[:], in_=pslice,
                                func=relu_fn, bias=bcols3_t[:, l:l + 1])
                            tp = hps.tile([128, F], f32, tag="hp")
                            nc.tensor.matmul(out=tp[:], lhsT=aggT[:],
                                             rhs=w_t[l + 1][:],
                                             start=True, stop=True)
                            nc.scalar.copy(out=dst_sl, in_=tp[:])
                        else:
                            aggT = aggp.tile([128, 128], f16, tag="aggT")
                            nc.scalar.add(out=aggT[:], in_=pslice,
                                          add=bcols3_t[:, 2:3])
                            tp = hps.tile([128, 128], f16, tag="hp",
                                          name="tppose")
                            nc.tensor.transpose(out=tp[:], in_=aggT[:],
                                                identity=ident[:])
                            nc.scalar.copy(out=dst_sl, in_=tp[:])
                    if l < 2 and w == WSPLIT - 1:
                        shard_write(0, WSPLIT * WPT)
                        allgather_piece(l + 1, 0)
                assert g == ncht
                if l < 2:
                    shard_write(WSPLIT * WPT, TILES)
                    # pre-issue next layer's class-0 gathers (piece A is
                    # already gathered) so desc-gen overlaps piece B
                    nxt = [tabsA[l + 1][:], tabsB[l + 1][:]]
                    for b in range(min(gcfg["pref"],
                                       -(-nch_cls[0] // BCH))):
                        prefetched[0][b] = issue_batch(0, b, nxt)
                    allgather_piece(l + 1, 1)

            # ---- mean pool, graph-major [G, F], reduce-scattered over graphs
            gpp = [gps.tile([128, F], f32, tag="gp", name=f"gp{gh}")
                   for gh in range(G // 128)]
            for t in range(TILES):
                for gh in range(G // 128):
                    nc.tensor.matmul(
                        out=gpp[gh][:],
                        lhsT=pools_t[:, t * G + gh * 128:t * G + gh * 128 + 128],
                        rhs=shb[:, t * F:(t + 1) * F],
                        start=(t == 0), stop=(t == TILES - 1))
            for gh in range(G // 128):
                gt = tmpp.tile([128, F], f32, tag="gtile")
                nc.scalar.copy(out=gt[:], in_=gpp[gh][:])
                nc.sync.dma_start(gt_in[gh * 128:(gh + 1) * 128, :], gt[:])
            nc.gpsimd.collective_compute(
                "ReduceScatter", mybir.AluOpType.add,
                replica_groups=[list(range(NCORES))],
                ins=[gt_in[:].opt()], outs=[rs_gt[:].opt()])

            # ---- MLP on this core's GS graphs ----
            rst = tmpp.tile([GS, 128], f32, tag="rst")
            nc.sync.dma_start(rst[:], rs_gt[:])
            rst16 = tmpp.tile([GS, 128], f16, tag="rst16")
            nc.vector.tensor_copy(rst16[:], rst[:])
            tpg = hps.tile([128, GS], f16, tag="hp", name="tpgt")
            nc.tensor.transpose(out=tpg[:], in_=rst16[:],
                                identity=ident[0:GS, 0:GS])
            gt16s = tmpp.tile([128, GS], f16, tag="gt16s")
            nc.scalar.copy(out=gt16s[:], in_=tpg[:])
            m16 = tmpp.tile([128, (NH // 128) * GS], f16, tag="m16")
            for h in range(NH // 128):
                m1p = gps.tile([128, GS], f32, tag="gp", name=f"m1p{h}")
                nc.tensor.matmul(out=m1p[:], lhsT=wm1f_t[:, h * 128:(h + 1) * 128],
                                 rhs=gt16s[:], start=True, stop=True)
                nc.scalar.activation(out=m16[:, h * GS:(h + 1) * GS], in_=m1p[:],
                                     func=relu_fn, bias=bm1f_t[:, h:h + 1])
            for oh in range(NO // 128):
                m2p = gps.tile([128, GS], f32, tag="gp", name=f"m2p{oh}")
                for h in range(NH // 128):
                    blk = (h * (NO // 128) + oh) * 128
                    nc.tensor.matmul(out=m2p[:],
                                     lhsT=wm2f_t[:, blk:blk + 128],
                                     rhs=m16[:, h * GS:(h + 1) * GS],
                                     start=(h == 0), stop=(h == NH // 128 - 1))
                ot = tmpp.tile([128, GS], f32, tag="ot")
                nc.scalar.add(out=ot[:], in_=m2p[:], add=bm2f_t[:, oh:oh + 1])
                nc.sync.dma_start(out[oh * 128:(oh + 1) * 128, :], ot[:])

    nc.compile()
    return nc


def _get_built(inputs, variant="full", gcfg=None):
    import hashlib
    h = hashlib.sha1()
    h.update(np.ascontiguousarray(inputs["edge_index"]).tobytes())
    h.update(np.ascontiguousarray(inputs["batch"]).tobytes())
    key = (variant,
           tuple(sorted((k, v.shape, str(v.dtype)) for k, v in inputs.items())),
           h.hexdigest())
    if key not in _cache:
        geom, in_maps = _host_prep(**inputs)
        nc = _build_bass(geom, variant, gcfg)
        _cache[key] = (geom, nc)
    else:
        geom, nc = _cache[key]
        _, in_maps = _host_prep(**inputs)
    return geom, nc, in_maps


def kernel(**inputs):
    inputs = {k: np.asarray(v) for k, v in inputs.items()}
    geom, nc, in_maps = _get_built(inputs)
    from concourse.bass_utils import run_bass_kernel_spmd
    res = run_bass_kernel_spmd(nc, in_maps, list(range(NCORES)))
    # per-core output is [NO, GS] (transposed); assemble to [G, NO]
    return np.concatenate([np.asarray(res.results[c]["out"]).T
                           for c in range(NCORES)], axis=0)


# revision 6
# speedup vs baseline: 1.4861x; 1.0291x over previous
"""GCN encoder (3x GCNConv + mean-pool + MLP) as an 8-core Trainium2 Bass kernel.

v4: host-precomputed scatter matrices + split AllGather + graph-sharded tail.

Sharding: nodes/edges partitioned by destination-node owner (8 shards).
Tables are W-premultiplied: tab0 = X@W0 (computed on device from per-core
transposed x shards), tab_{l+1} = relu(agg_l + b_l) @ W_{l+1}. Each layer
table lives as TWO contiguous tensors: piece A (every core's first RSPL
shard rows, 8*RSPL = 32768 so gather indices fit int16) and piece B (the
rest). Piece A AllGathers mid-layer (after window WSPLIT-1), hidden under
the remaining windows' compute; only the smaller piece B is exposed at the
layer boundary. Edges are classed by which piece their source row is in.
Per layer: per-edge source rows are gathered from the table (fp16 DRAM) with
dma_gather; the one-hot scatter matrices S (identical across layers, graph-
dependent only) are precomputed on the HOST in fp8e4m3 and streamed from DRAM
in groups, so the vector engine does no per-chunk work. Scatter-add happens
via PE matmul (lhsT = gathered messages fp16, rhs = S fp8). Self-loop and
mean-pool selection matrices are host-built fp16 and SBUF-resident. The psum
drain fuses bias+relu on the ACT engine in feature-major layout, and the
next-table matmul transposes to node-major for free. Final layer transposes
via PE; mean-pool produces graph-major [G, F] partials which are
ReduceScattered over graphs (no AllReduce); each core then runs the full MLP
on its 32 graphs with resident weights and returns a transposed [NO, 32]
block that the host reassembles.
"""

import numpy as np

NCORES = 8
F = 128            # hidden width
G = 256            # number of graphs
NH = 512           # MLP hidden
NO = 256           # MLP out
CH = 128           # edges per chunk
BATCH_CH = 16      # chunks per dma_gather batch
WINW = 256         # dst nodes per PSUM accumulation window
SGRP = 8           # chunks per S-matrix stream DMA
WSPLIT = 16        # windows whose rows go in AllGather piece A
XT_FP8 = True      # ship x shards as fp8e4m3 (halves xt upload)

_cache = {}


def _host_prep(x, edge_index, batch, W0, b0, W1, b1, W2, b2, Wm1, bm1, Wm2, bm2):
    import ml_dtypes
    f8 = ml_dtypes.float8_e4m3

    N = x.shape[0]
    FI = x.shape[1]
    SH = -(-N // (NCORES * 128)) * 128      # shard size (nodes), 128-multiple
    NP = SH * NCORES
    TILES = SH // 128
    NWIN = -(-SH // WINW)
    RSPL = WSPLIT * WINW                    # shard rows in AllGather piece A
    LO = NCORES * RSPL                      # rows in table piece A (<= 32768)
    HI = NP - LO                            # rows in table piece B
    NHS = NH // NCORES                      # MLP hidden slice per core
    GS = G // NCORES                        # output graphs per core

    src = np.asarray(edge_index[0], dtype=np.int64)
    dst = np.asarray(edge_index[1], dtype=np.int64)
    deg = (np.bincount(np.concatenate([dst, np.arange(N, dtype=np.int64)]),
                       minlength=N).astype(np.float32))
    dis = np.where(deg > 0, 1.0 / np.sqrt(np.maximum(deg, 1.0)), 0.0).astype(np.float32)
    norm = dis[src] * dis[dst]

    # per-core edge selection, ordered by (window, class, dst).
    # class 0: src local-offset < RSPL (table piece A, gathered early);
    # class 1: src local-offset >= RSPL (table piece B).
    per_core = []
    for c in range(NCORES):
        base = c * SH
        sel = (dst >= base) & (dst < base + SH)
        es = src[sel].astype(np.int64)
        ed = (dst[sel] - base).astype(np.int64)
        en = norm[sel]
        sc, sl_ = es // SH, es % SH
        cl = (sl_ >= RSPL).astype(np.int64)
        ei = np.where(cl == 0, sc * RSPL + sl_, sc * (SH - RSPL) + (sl_ - RSPL))
        wi = ed // WINW
        order = np.lexsort((ed, cl, wi))
        per_core.append((ei[order], ed[order], en[order], cl[order], wi[order]))

    # chunk counts per (window, class), equalized across cores
    counts = np.zeros((NCORES, NWIN, 2), dtype=np.int64)
    for c in range(NCORES):
        _, _, _, cl, wi = per_core[c]
        for cls in (0, 1):
            counts[c, :, cls] = np.bincount(wi[cl == cls], minlength=NWIN)
    nch = -(-counts.max(axis=0) // CH)  # [NWIN, 2] chunks
    nch_cls = nch.sum(axis=0)          # total chunks per class
    ncht = int(nch.sum())

    # shared program schedule: windows -> list of (cls, cid)
    schedule = []
    cid_ctr = [0, 0]
    for w in range(NWIN):
        lst = []
        for cls in (0, 1):
            for _ in range(int(nch[w, cls])):
                lst.append((cls, cid_ctr[cls]))
                cid_ctr[cls] += 1
        schedule.append(lst)

    # per-core streams: compact idx [16, nch_cls*8] int16;
    # host-built scatter matrices s_all [128, ncht*WINW] fp8
    idx_streams = [[], []]
    s_alls = []
    for c in range(NCORES):
        es, ed, en, cl, wi = per_core[c]
        idx_parts = [[], []]
        s_all = np.zeros((128, ncht * WINW), dtype=np.float32)
        g = 0
        pos = 0
        for w in range(NWIN):
            for cls in (0, 1):
                n_e = int(counts[c, w, cls])
                tot = int(nch[w, cls]) * CH
                ge, gd, gn = es[pos:pos + n_e], ed[pos:pos + n_e], en[pos:pos + n_e]
                pos += n_e
                pad = tot - n_e
                iv = np.concatenate([ge, np.zeros(pad, np.int64)])
                dl = np.concatenate([gd - w * WINW, np.full(pad, -1, np.int64)])
                nr = np.concatenate([gn, np.zeros(pad, np.float32)])
                idx_parts[cls].append(iv.astype(np.int16))
                for k in range(tot // CH):
                    sl = slice(k * CH, (k + 1) * CH)
                    dlk, nrk = dl[sl], nr[sl]
                    valid = dlk >= 0
                    rows = np.nonzero(valid)[0]
                    s_all[rows, g * WINW + dlk[valid]] = nrk[valid]
                    g += 1
        assert g == ncht
        s_alls.append(s_all.astype(f8))
        for cls in (0, 1):
            arr = (np.concatenate(idx_parts[cls]) if idx_parts[cls]
                   else np.zeros(0, np.int16))
            assert arr.size == nch_cls[cls] * CH
            if arr.size:
                wrapped = arr.reshape(-1, 16).T       # [16, nch_cls*8]
            else:
                wrapped = np.zeros((16, 8), np.int16)  # dummy
            idx_streams[cls].append(np.ascontiguousarray(wrapped))

    # resident selection matrices: self-loop diag + mean-pool, per tile
    cnt = np.bincount(batch.astype(np.int64), minlength=G).astype(np.float32)
    invc_all = (1.0 / np.maximum(cnt, 1.0))[batch.astype(np.int64)]
    selfnr_all = dis * dis
    selfs_l, pools_l = [], []
    for c in range(NCORES):
        lo_n = c * SH
        hi_n = min((c + 1) * SH, N)
        nreal = max(0, hi_n - lo_n)
        selfs = np.zeros((128, TILES * 128), dtype=np.float16)
        pools = np.zeros((128, TILES * G), dtype=np.float16)
        for t in range(TILES):
            for p in range(128):
                v = t * 128 + p
                if v < nreal:
                    selfs[p, t * 128 + p] = selfnr_all[lo_n + v]
                    bgi = int(batch[lo_n + v])
                    pools[p, t * G + bgi] = invc_all[lo_n + v]
        selfs_l.append(selfs)
        pools_l.append(pools)

    # full MLP weights, graph-sharded tail: bm1f/bm2f wrapped per 128-block
    wm2f = np.zeros((128, (NH // 128) * (NO // 128) * 128), np.float16)
    for h in range(NH // 128):
        for oh in range(NO // 128):
            wm2f[:, (h * (NO // 128) + oh) * 128:
                 (h * (NO // 128) + oh + 1) * 128] = \
                Wm2[h * 128:(h + 1) * 128, oh * 128:(oh + 1) * 128]
    consts = {
        "w0": W0.astype(np.float16),                     # [FI, F]
        "w1": W1.astype(np.float16), "w2": W2.astype(np.float16),
        "bcols3": np.stack([b0, b1, b2], axis=1).astype(np.float32),  # [F, 3]
        "ident": np.eye(128, dtype=np.float16),
        "wm1f": np.ascontiguousarray(Wm1).astype(np.float16),   # [F, NH]
        "bm1f": np.ascontiguousarray(
            np.asarray(bm1).reshape(NH // 128, 128).T).astype(np.float32),
        "wm2f": wm2f,
        "bm2f": np.ascontiguousarray(
            np.asarray(bm2).reshape(NO // 128, 128).T).astype(np.float32),
    }
    xt_np = f8 if XT_FP8 else np.float16
    in_maps = []
    for c in range(NCORES):
        m = dict(consts)
        lo = c * SH
        hi = min((c + 1) * SH, N)
        xt = np.zeros((FI, SH), dtype=xt_np)
        xt[:, :hi - lo] = x[lo:hi].T.astype(xt_np)
        m["xt"] = np.ascontiguousarray(xt)
        m["idxlo"] = idx_streams[0][c]
        m["idxhi"] = idx_streams[1][c]
        m["s_all"] = s_alls[c]
        m["selfs"] = selfs_l[c]
        m["pools"] = pools_l[c]
        in_maps.append(m)

    geom = dict(N=N, FI=FI, NP=NP, SH=SH, TILES=TILES, NWIN=NWIN, LO=LO, HI=HI,
                NHS=NHS, GS=GS, nch=nch, nch_cls=[int(v) for v in nch_cls],
                ncht=ncht, schedule=schedule)
    return geom, in_maps


def _build_bass(geom, variant="full", gcfg=None):
    import concourse.bass as bass
    import concourse.tile as tile
    from concourse import bacc, mybir

    gcfg = dict(dict(batch=BATCH_CH, sp=False, nq=4, qg=True,
                     mbufs=8, sbufs=6, wbufs=3, prep=False, pref=0),
                **(gcfg or {}))
    BCH = gcfg["batch"]

    f16, f32, i16 = mybir.dt.float16, mybir.dt.float32, mybir.dt.int16
    f8 = mybir.dt.float8e4
    fxt = f8 if XT_FP8 else f16
    FI, NP, SH, TILES, NWIN = (geom["FI"], geom["NP"], geom["SH"],
                               geom["TILES"], geom["NWIN"])
    LO, HI, NHS, GS = geom["LO"], geom["HI"], geom["NHS"], geom["GS"]
    nch, nch_cls, ncht = geom["nch"], geom["nch_cls"], geom["ncht"]
    schedule = geom["schedule"]
    NSG = -(-ncht // SGRP)  # number of S stream groups

    nc = bacc.Bacc("TRN2", target_bir_lowering=False, debug=False,
                   num_devices=NCORES, num_swdge_queues=gcfg["nq"])

    xt = nc.dram_tensor("xt", [FI, SH], fxt, kind="ExternalInput")
    idxlo = nc.dram_tensor("idxlo", [16, max(nch_cls[0] * 8, 8)], i16,
                           kind="ExternalInput")
    idxhi = nc.dram_tensor("idxhi", [16, max(nch_cls[1] * 8, 8)], i16,
                           kind="ExternalInput")
    s_all = nc.dram_tensor("s_all", [128, ncht * WINW], f8, kind="ExternalInput")
    selfs = nc.dram_tensor("selfs", [128, TILES * 128], f16, kind="ExternalInput")
    pools = nc.dram_tensor("pools", [128, TILES * G], f16, kind="ExternalInput")
    w0 = nc.dram_tensor("w0", [FI, F], f16, kind="ExternalInput")
    w1 = nc.dram_tensor("w1", [F, F], f16, kind="ExternalInput")
    w2 = nc.dram_tensor("w2", [F, F], f16, kind="ExternalInput")
    bcols3 = nc.dram_tensor("bcols3", [F, 3], f32, kind="ExternalInput")
    ident_d = nc.dram_tensor("ident", [128, 128], f16, kind="ExternalInput")
    wm1f = nc.dram_tensor("wm1f", [F, NH], f16, kind="ExternalInput")
    bm1f = nc.dram_tensor("bm1f", [128, NH // 128], f32, kind="ExternalInput")
    wm2f = nc.dram_tensor("wm2f", [128, (NH // 128) * (NO // 128) * 128], f16,
                          kind="ExternalInput")
    bm2f = nc.dram_tensor("bm2f", [128, NO // 128], f32, kind="ExternalInput")
    out = nc.dram_tensor("out", [NO, GS], f32, kind="ExternalOutput")

    shard_d = nc.dram_tensor("shard_d", [SH, F], f16)
    RSPL = LO // NCORES
    tabsA = [nc.dram_tensor(f"tabA{l}", [LO, F], f16, addr_space="Shared")
             for l in range(3)]
    tabsB = [nc.dram_tensor(f"tabB{l}", [HI, F], f16, addr_space="Shared")
             for l in range(3)]
    gt_in = nc.dram_tensor("gt_in", [G, 128], f16)
    rs_gt = nc.dram_tensor("rs_gt", [GS, 128], f16)

    shb = nc.alloc_sbuf_tensor("shb", [128, TILES * F], f16)

    relu_fn = mybir.ActivationFunctionType.Relu

    with tile.TileContext(nc) as tc:
        with (
            tc.tile_pool(name="res", bufs=1) as res,
            tc.tile_pool(name="msg", bufs=gcfg["mbufs"]) as msgp,
            tc.tile_pool(name="sst", bufs=gcfg["sbufs"]) as sstp,
            tc.tile_pool(name="agg", bufs=2) as aggp,
            tc.tile_pool(name="tmp", bufs=2) as tmpp,
            tc.tile_pool(name="wps", bufs=gcfg["wbufs"], space="PSUM") as wps,
            tc.tile_pool(name="hps", bufs=2, space="PSUM") as hps,
            tc.tile_pool(name="gps", bufs=2, space="PSUM") as gps,
        ):
            # ---- resident loads ----
            def load(t_dram, shape, dtype):
                t = res.tile(shape, dtype, tag=t_dram.name)
                nc.sync.dma_start(t[:], t_dram[:])
                return t

            idx_t = []
            for cls, t_dram in ((0, idxlo), (1, idxhi)):
                w = max(nch_cls[cls] * 8, 8)
                t = res.tile([128, w], i16, tag=f"idx{cls}", name=f"idxt{cls}")
                for k in range(8):
                    nc.sync.dma_start(t[16 * k:16 * (k + 1), :], t_dram[:])
                idx_t.append(t)
            xt_t = load(xt, [FI, SH], fxt)
            selfs_t = load(selfs, [128, TILES * 128], f16)
            pools_t = load(pools, [128, TILES * G], f16)
            w0_t = load(w0, [FI, F], f16)
            w_t = {1: load(w1, [F, F], f16), 2: load(w2, [F, F], f16)}
            bcols3_t = load(bcols3, [F, 3], f32)
            ident = load(ident_d, [128, 128], f16)
            wm1f_t = load(wm1f, [F, NH], f16)
            bm1f_t = load(bm1f, [128, NH // 128], f32)
            wm2f_t = load(wm2f, [128, (NH // 128) * (NO // 128) * 128], f16)
            bm2f_t = load(bm2f, [128, NO // 128], f32)
            dma_sems = None
            if gcfg["prep"]:
                dma_sems = [nc.alloc_semaphore(f"gsem{q}")
                            for q in range(gcfg["nq"])]
                for s_ in dma_sems:
                    nc.gpsimd.dma_reset(range(s_.num, s_.num + 1))
                    nc.gpsimd.sem_clear(range(s_.num, s_.num + 1))

            # ---- T0 = X @ W0 (per-shard), node-major into shb ----
            WSPLIT = LO // (NCORES * WINW)  # windows in AllGather piece A
            WPT = WINW // 128

            def shard_write(lo_t, hi_t):
                nc.sync.dma_start(
                    shard_d[lo_t * 128:hi_t * 128, :]
                    .rearrange("(t p) f -> p t f", p=128),
                    shb[:, lo_t * F:hi_t * F]
                    .rearrange("p (t f) -> p t f", f=F))

            def allgather_piece(l_next, piece):
                if piece == 0:
                    ins_ap, outs_ap = shard_d[0:RSPL, :], tabsA[l_next][:]
                else:
                    ins_ap, outs_ap = shard_d[RSPL:SH, :], tabsB[l_next][:]
                nc.gpsimd.collective_compute(
                    "AllGather", mybir.AluOpType.bypass,
                    replica_groups=[list(range(NCORES))],
                    ins=[ins_ap.opt()], outs=[outs_ap.opt()])

            for t in range(TILES):
                t0p = hps.tile([128, F], f32, tag="hp")
                nc.tensor.matmul(out=t0p[:], lhsT=xt_t[:, 128 * t:128 * (t + 1)],
                                 rhs=w0_t[:], start=True, stop=True)
                nc.scalar.copy(out=shb[:, t * F:(t + 1) * F], in_=t0p[:])
                if t == WSPLIT * WPT - 1:
                    shard_write(0, WSPLIT * WPT)
                    allgather_piece(0, 0)
            shard_write(WSPLIT * WPT, TILES)
            allgather_piece(0, 1)

            # ---- 3 GCN layers ----
            gctr = [0]  # global gather counter for queue round-robin
            prefetched = [{}, {}]  # cls -> {batch: msg tile}, for next layer

            def issue_batch(cls, b, tbl_ap_l):
                nb = min(BCH, nch_cls[cls] - b * BCH)
                mt = msgp.tile([128, BCH, F], f16, tag=f"msg{cls}")
                qn = (gctr[0] if gcfg["qg"] else b) % gcfg["nq"]
                gctr[0] += 1
                if gcfg["prep"]:
                    nc.gpsimd.dma_gather(
                        mt[:, :nb, :], tbl_ap_l[cls],
                        idx_t[cls][:, b * (BCH * 8):b * (BCH * 8) + nb * 8],
                        nb * CH, nb * CH, F,
                        single_packet=gcfg["sp"], prepare_only=True,
                        sem=dma_sems[qn], queue_num=qn)
                    nc.gpsimd.trigger_dma(count=None, queue_num=qn)
                else:
                    nc.gpsimd.dma_gather(
                        mt[:, :nb, :], tbl_ap_l[cls],
                        idx_t[cls][:, b * (BCH * 8):b * (BCH * 8) + nb * 8],
                        nb * CH, nb * CH, F,
                        single_packet=gcfg["sp"], queue_num=qn)
                return mt

            for l in range(3):
                tbl_ap = [tabsA[l][:], tabsB[l][:]]
                issued = [-1, -1]
                cur = [None, None]
                sg_cur = [-1]
                st_cur = [None]
                g = 0
                WPT = WINW // 128
                for w in range(NWIN):
                    width = min(WINW, SH - w * WINW)
                    chunks = schedule[w]
                    ops = [("c", x) for x in chunks]
                    selfops = [("s", sub) for sub in range(width // 128)]
                    ops = (ops[:1] + selfops + ops[1:]) if ops else selfops
                    ps = wps.tile([128, WINW], f32, tag="wps")
                    no_chunks = not chunks
                    for j, op in enumerate(ops):
                        first, last = (j == 0), (j == len(ops) - 1)
                        if op[0] == "s":
                            sub = op[1]
                            t_idx = w * WPT + sub
                            nc.tensor.matmul(
                                out=ps[:, sub * 128:sub * 128 + 128],
                                lhsT=shb[:, t_idx * F:(t_idx + 1) * F],
                                rhs=selfs_t[:, t_idx * 128:(t_idx + 1) * 128],
                                start=(True if no_chunks else False),
                                stop=last, skip_group_check=True)
                            continue
                        cls, cid = op[1]
                        b, slab = divmod(cid, BCH)
                        if b != issued[cls]:
                            if b in prefetched[cls]:
                                mt = prefetched[cls].pop(b)
                            else:
                                mt = issue_batch(cls, b, tbl_ap)
                            issued[cls] = b
                            cur[cls] = mt
                        sg = g // SGRP
                        if sg != sg_cur[0]:
                            ngc = min(SGRP, ncht - sg * SGRP)
                            st = sstp.tile([128, SGRP * WINW], f8, tag="sst")
                            nc.sync.dma_start(
                                st[:, :ngc * WINW],
                                s_all[:, sg * SGRP * WINW:
                                      (sg * SGRP + ngc) * WINW])
                            sg_cur[0] = sg
                            st_cur[0] = st
                        so = (g - sg * SGRP) * WINW
                        nc.tensor.matmul(
                            out=ps[:, :width], lhsT=cur[cls][:, slab, :],
                            rhs=st_cur[0][:, so:so + width],
                            start=first, stop=last)
                        g += 1
                    # drain window: fuse bias (+relu) on ACT per subtile
                    for sub in range(width // 128):
                        t_idx = w * WPT + sub
                        pslice = ps[:, sub * 128:(sub + 1) * 128]
                        dst_sl = shb[:, t_idx * F:(t_idx + 1) * F]
                        if l < 2:
                            aggT = aggp.tile([128, 128], f16, tag="aggT")
                            nc.scalar.activation(
                                out=aggT[:], in_=pslice,
                                func=relu_fn, bias=bcols3_t[:, l:l + 1])
                            tp = hps.tile([128, F], f32, tag="hp")
                            nc.tensor.matmul(out=tp[:], lhsT=aggT[:],
                                             rhs=w_t[l + 1][:],
                                             start=True, stop=True)
                            nc.scalar.copy(out=dst_sl, in_=tp[:])
                        else:
                            aggT = aggp.tile([128, 128], f16, tag="aggT")
                            nc.scalar.add(out=aggT[:], in_=pslice,
                                          add=bcols3_t[:, 2:3])
                            tp = hps.tile([128, 128], f16, tag="hp",
                                          name="tppose")
                            nc.tensor.transpose(out=tp[:], in_=aggT[:],
                                                identity=ident[:])
                            nc.scalar.copy(out=dst_sl, in_=tp[:])
                    if l < 2 and w == WSPLIT - 1:
                        shard_write(0, WSPLIT * WPT)
                        allgather_piece(l + 1, 0)
                assert g == ncht
                if l < 2:
                    shard_write(WSPLIT * WPT, TILES)
                    # pre-issue next layer's class-0 gathers (piece A is
                    # already gathered) so desc-gen overlaps piece B
                    nxt = [tabsA[l + 1][:], tabsB[l + 1][:]]
                    for b in range(min(gcfg["pref"],
                                       -(-nch_cls[0] // BCH))):
                        prefetched[0][b] = issue_batch(0, b, nxt)
                    allgather_piece(l + 1, 1)

            # ---- mean pool, graph-major [G, F], reduce-scattered over graphs
            gpp = [gps.tile([128, F], f32, tag="gp", name=f"gp{gh}")
                   for gh in range(G // 128)]
            for t in range(TILES):
                for gh in range(G // 128):
                    nc.tensor.matmul(
                        out=gpp[gh][:],
                        lhsT=pools_t[:, t * G + gh * 128:t * G + gh * 128 + 128],
                        rhs=shb[:, t * F:(t + 1) * F],
                        start=(t == 0), stop=(t == TILES - 1))
            for gh in range(G // 128):
                gt = tmpp.tile([128, F], f16, tag="gtile")
                nc.scalar.copy(out=gt[:], in_=gpp[gh][:])
                nc.sync.dma_start(gt_in[gh * 128:(gh + 1) * 128, :], gt[:])
            nc.gpsimd.collective_compute(
                "ReduceScatter", mybir.AluOpType.add,
                replica_groups=[list(range(NCORES))],
                ins=[gt_in[:].opt()], outs=[rs_gt[:].opt()])

            # ---- MLP on this core's GS graphs ----
            rst = tmpp.tile([GS, 128], f16, tag="rst")
            nc.sync.dma_start(rst[:], rs_gt[:])
            tpg = hps.tile([128, GS], f16, tag="hp", name="tpgt")
            nc.tensor.transpose(out=tpg[:], in_=rst[:],
                                identity=ident[0:GS, 0:GS])
            gt16s = tmpp.tile([128, GS], f16, tag="gt16s")
            nc.scalar.copy(out=gt16s[:], in_=tpg[:])
            m16 = tmpp.tile([128, (NH // 128) * GS], f16, tag="m16")
            for h in range(NH // 128):
                m1p = gps.tile([128, GS], f32, tag="gp", name=f"m1p{h}")
                nc.tensor.matmul(out=m1p[:], lhsT=wm1f_t[:, h * 128:(h + 1) * 128],
                                 rhs=gt16s[:], start=True, stop=True)
                nc.scalar.activation(out=m16[:, h * GS:(h + 1) * GS], in_=m1p[:],
                                     func=relu_fn, bias=bm1f_t[:, h:h + 1])
            for oh in range(NO // 128):
                m2p = gps.tile([128, GS], f32, tag="gp", name=f"m2p{oh}")
                for h in range(NH // 128):
                    blk = (h * (NO // 128) + oh) * 128
                    nc.tensor.matmul(out=m2p[:],
                                     lhsT=wm2f_t[:, blk:blk + 128],
                                     rhs=m16[:, h * GS:(h + 1) * GS],
                                     start=(h == 0), stop=(h == NH // 128 - 1))
                ot = tmpp.tile([128, GS], f32, tag="ot")
                nc.scalar.add(out=ot[:], in_=m2p[:], add=bm2f_t[:, oh:oh + 1])
                nc.sync.dma_start(out[oh * 128:(oh + 1) * 128, :], ot[:])

    nc.compile()
    return nc


def _get_built(inputs, variant="full", gcfg=None):
    import hashlib
    h = hashlib.sha1()
    h.update(np.ascontiguousarray(inputs["edge_index"]).tobytes())
    h.update(np.ascontiguousarray(inputs["batch"]).tobytes())
    key = (variant,
           tuple(sorted((k, v.shape, str(v.dtype)) for k, v in inputs.items())),
           h.hexdigest())
    if key not in _cache:
        geom, in_maps = _host_prep(**inputs)
        nc = _build_bass(geom, variant, gcfg)
        _cache[key] = (geom, nc)
    else:
        geom, nc = _cache[key]
        _, in_maps = _host_prep(**inputs)
    return geom, nc, in_maps


def kernel(**inputs):
    inputs = {k: np.asarray(v) for k, v in inputs.items()}
    geom, nc, in_maps = _get_built(inputs)
    from concourse.bass_utils import run_bass_kernel_spmd
    res = run_bass_kernel_spmd(nc, in_maps, list(range(NCORES)))
    # per-core output is [NO, GS] (transposed); assemble to [G, NO]
    return np.concatenate([np.asarray(res.results[c]["out"]).T
                           for c in range(NCORES)], axis=0)
